# revision 1
# baseline (speedup 1.0000x reference)
"""Trainium2 Bass kernel for nn_DenseCapsuleLayer.

Reference computation:
    u_hat[b, j, k, n] = sum_m W[0, j, idx[b,k], n, m] * x[b, idx[b,k], m]
with idx[b, :] the ascending indices of the NZC=1152 non-zero child capsules
of batch b (x is zero elsewhere).

Strategy (8 NeuronCores, 2-way parent-capsule x 4-way batch mesh):
  * Core c owns j in [16*(c%2), 16*(c%2)+16) and b in [8*(c//2), 8*(c//2)+8).
  * Each core computes the DENSE map u_full[b, i, jl, n] for ALL i (x is zero
    at non-selected i, so u_full there is zero and is discarded); the
    select/compaction gather over i and the unshard/concat happen on the
    host.
  * Per 16-wide child-capsule chunk g (i = 16g+il), the PE computes
        out[(il,bl), (jl,n)] = sum_m x[b, 16g+il, m] * W[j, 16g+il, n, m]
    as ONE K=128 matmul: the stationary operand is a [128,128]
    block-diagonal packing of the core's x slice (8 batches) built ON DEVICE
    by a broadcast multiply with a static 0/1 mask (x ships compact, 8
    floats per row), the moving operand is the core's W slice pre-transposed
    to [i, m, (jl,n)] (256 free columns).  All matmuls keep base partition
    0 (mixing stationary base partitions crashes this device).

Toolchain constraints: every lowered instruction accepts ONE sync-wait
command and Tile emits a wait per dependency, so dummy ops absorb all but
one dependency per real instruction, SP nops "park" the kernel-tail drain's
wait list, and a BIR post-pass drops DMA waits that are provably implied by
the single wait that is kept.
"""

import numpy as np

B, I, J, M, N = 32, 2304, 32, 8, 16
NZC = I // 2
NCORES = 8
JL = J // 2               # parent capsules per core (16)
JN = JL * N               # 256
BL = B // 4               # batches per core (8)
NCHUNK = I // 16          # 144 chunks of 16 child capsules
NSTRIP = 8
CH_PER_STRIP = NCHUNK // NSTRIP  # 18
PAD = 4                   # o_sb pad elements (see dummy B)

_CACHE = {}


def _build_program():
    import concourse.bass as bass
    import concourse.mybir as mybir
    import concourse.tile as tile

    f32 = mybir.dt.float32
    nc = bass.Bass()

    # wb[g, (il,m), 0:256]   = W[j, 16g+il, n, m]  (moving operand)
    # wb[g, (il,m), 256:264] = x[b, 16g+il, m] for the core's 8 batches
    wb = nc.declare_dram_parameter("wb", [NCHUNK, 128, JN + BL], f32,
                                   isOutput=False)
    # msk[(il,m), (il',bl)] = 1.0 iff il == il'
    msk = nc.declare_dram_parameter("msk", [128, 128], f32, isOutput=False)
    u = nc.declare_dram_parameter(
        "u", [128, NSTRIP * (CH_PER_STRIP * JN + PAD)], f32, isOutput=True
    )
    SJN = CH_PER_STRIP * JN + PAD

    with tile.TileContext(nc, pool_alloc_mode="queue") as tc:
        with (
            tc.tile_pool(name="wpool", bufs=3) as wpool,
            tc.tile_pool(name="bdpool", bufs=3) as bdpool,
            tc.tile_pool(name="opool", bufs=3) as opool,
            tc.tile_pool(name="ppool", bufs=6, space="PSUM") as ppool,
            tc.tile_pool(name="dpool", bufs=1, space="PSUM") as dpool,
            tc.tile_pool(name="zpool", bufs=1) as zpool,
        ):
            # dmy: PE dummy-matmul targets (each column written exactly once)
            # sig: written by DVE right after each PSUM->SBUF copy; read by
            #      DVE dummy A to advance the DVE clock across strips
            dmy = dpool.tile([1, 160], f32, tag="d")
            sig = zpool.tile([32, 160], f32, tag="sig")
            sig2 = zpool.tile([1, 128], f32, tag="sig2")
            z_sb = zpool.tile([128, PAD], f32, tag="z")
            mask_t = zpool.tile([128, 128], f32, tag="msk")
            nc.vector.memset(z_sb[:, :], 0.0)
            d_msk = nc.sync.dma_start(out=mask_t[:, :], in_=msk[:, :])
            # absorbs the mask-load wait on the DVE queue
            nc.vector.tensor_copy(sig2[0:1, 120:121], mask_t[0:1, 0:1])
            mask3 = mask_t.rearrange("p (r c) -> p r c", r=16)

            all_dmas = [d_msk]
            cps = []

            def park_wait(dep, prev=None):
                w = nc.sync.nop(nofuse=True, hint="park")
                tile.add_dep_helper(w.ins, dep.ins, sync=True, reason="park")
                if prev is not None:
                    tile.add_dep_helper(w.ins, prev.ins, sync=False, reason="ord")
                return w

            for s in range(NSTRIP):
                glo = s * CH_PER_STRIP
                w_sb = wpool.tile([128, CH_PER_STRIP, JN + BL], f32, tag="w")
                # carries (s>=3) the w-slot WAR: PE readers of strip s-3 (the
                # DVE readers and the slot WAW are implied; post-pass below)
                d_in = nc.sync.dma_start(
                    out=w_sb[:, :, :],
                    in_=wb[glo : glo + CH_PER_STRIP].rearrange("g p c -> p g c"),
                )
                all_dmas.append(d_in)
                # absorb the strip-DMA wait on the PE queue...
                sdum = nc.tensor.matmul(
                    dmy[0:1, s : s + 1],
                    w_sb[0:32, 0, 0:1],
                    w_sb[0:32, 0, 0:1],
                    start=True,
                    stop=True,
                )
                # ...and on the DVE queue (for the bd builder muls)
                sdumv = nc.vector.tensor_copy(
                    sig2[0:1, 8 + s : 9 + s], w_sb[0:1, 0, JN : JN + 1]
                )
                o_sb = opool.tile([128, CH_PER_STRIP * JN + PAD], f32, tag="o")
                bdt = bdpool.tile([128, CH_PER_STRIP, 128], f32, tag="bd")
                adum = None
                if s >= 3:
                    # dummy A: advances the DVE clock past all of strip s-3's
                    # copies (covers copy/bd-mul WAWs and dummy B's pad WAW)
                    adum = nc.vector.tensor_copy(
                        sig2[0:1, 96 + s : 97 + s], sig[0:1, s - 3 : s - 2]
                    )
                # dummy B: pad write carries the o_sb slot-reuse WAR (the
                # out-DMA of strip s-3 read the pad too, so the WAR re-forms)
                bdum = nc.vector.tensor_copy(
                    o_sb[:, CH_PER_STRIP * JN : CH_PER_STRIP * JN + PAD],
                    z_sb[:, :],
                )
                if adum is not None:
                    tile.add_dep_helper(
                        bdum.ins, adum.ins, sync=False, reason="A before B"
                    )
                for gl in range(CH_PER_STRIP):
                    gg = s * CH_PER_STRIP + gl  # global chunk index
                    # build the block-diagonal stationary on device:
                    # bdt[p, (il', bl)] = x[p-row] * mask[p, (il', bl)]
                    mul = nc.vector.tensor_mul(
                        bdt[:, gl, :].rearrange("p (r c) -> p r c", r=16),
                        w_sb[:, gl : gl + 1, JN : JN + BL].broadcast_to(
                            [128, 16, BL]
                        ),
                        mask3,
                    )
                    pair = gg // 2
                    if pair >= 6:
                        # the bank-WAR coverage via gdum's mul-tick needs this
                        # mul scheduled AFTER the copy that frees the pair's
                        # PSUM bank (6 pairs back) on the DVE queue
                        tile.add_dep_helper(
                            mul.ins, cps[pair - 6].ins, sync=False,
                            reason="mul after bank-freeing copy",
                        )
                    if gl == 0:
                        tile.add_dep_helper(
                            mul.ins, sdumv.ins, sync=False,
                            reason="dve strip dummy before muls",
                        )
                        if adum is not None:
                            tile.add_dep_helper(
                                mul.ins, adum.ins, sync=False,
                                reason="A before first mul",
                            )
                    if gl % 2 == 0:
                        ps = ppool.tile([128, 2, JN], f32, tag="ps")
                    # absorbs (on PE) the RAW wait on the bd mul, which also
                    # covers the PSUM-bank WAR (the freeing copy ran earlier
                    # on the same DVE queue)
                    gdum = nc.tensor.matmul(
                        dmy[0:1, 8 + gg : 9 + gg],
                        bdt[0:32, gl, 0:1],
                        bdt[0:32, gl, 0:1],
                        start=True,
                        stop=True,
                    )
                    mm = nc.tensor.matmul(
                        ps[:, gl % 2, :],
                        bdt[:, gl, :],
                        w_sb[0:128, gl, 0:JN],
                        start=True,
                        stop=True,
                    )
                    tile.add_dep_helper(
                        mm.ins, gdum.ins, sync=False, reason="gdum before MM"
                    )
                    if gl == 0:
                        tile.add_dep_helper(
                            mm.ins, sdum.ins, sync=False,
                            reason="strip dummy before first MM",
                        )
                    # one copy per chunk PAIR; carries only its RAW wait
                    if gl % 2 == 1:
                        cp = nc.vector.tensor_copy(
                            o_sb[:, (gl - 1) * JN : (gl + 1) * JN],
                            ps.rearrange("p a b -> p (a b)"),
                        )
                        tile.add_dep_helper(
                            cp.ins, bdum.ins, sync=False, reason="B before copies"
                        )
                        cps.append(cp)
                    last_mm = mm
                # sig write (one per strip): RAW on the strip's last copy
                # keeps DVE ordering; read by dummy A two strips later
                last_sigw = nc.vector.tensor_copy(
                    sig[0:32, s : s + 1],
                    o_sb[0:32, (CH_PER_STRIP - 1) * JN : (CH_PER_STRIP - 1) * JN + 1],
                )
                # carries only its DVE wait; lane wait dropped by post-pass.
                # Issued from the ACT sequencer so input (SP) and output
                # DMA streams overlap.
                d_out = nc.scalar.dma_start(
                    out=u[:, s * SJN : (s + 1) * SJN], in_=o_sb[:, :]
                )
                all_dmas.append(d_out)
            # tail parking: cover the last 8 DMAs + engine tails so the
            # kernel-tail drain has at most one wait left
            prev = None
            for d in all_dmas + [last_mm, last_sigw]:
                prev = park_wait(d, prev)

    # Single-wait legalization: keep the strongest wait per DMA (PE if
    # present, else DVE) — the dropped DMAHW/DVE waits are implied by it
    # through the dummy-op ordering chains (the kept tick is only reached
    # after the dropped dependencies completed).
    import concourse.mybir as mybir2

    for blk in nc.m.functions[0].blocks:
        for inst in blk.instructions:
            si = inst.sync_info
            if si is None or not si.on_wait or len(si.on_wait) < 2:
                continue
            if type(inst).__name__ != "InstDMACopy":
                raise RuntimeError(f"unexpected multi-wait {inst.name}")
            pe = [w for w in si.on_wait if w.ant_name.startswith("PE")]
            dve = [w for w in si.on_wait if w.ant_name.startswith("DVE")]
            dma = [w for w in si.on_wait if w.ant_name.startswith("DMAHW")]
            if len(pe) + len(dve) + len(dma) != len(si.on_wait):
                raise RuntimeError(f"unexpected wait mix on {inst.name}")
            keep = pe[:1] or dve[:1]
            if len(keep) != 1:
                raise RuntimeError(f"no engine wait to keep on {inst.name}")
            inst.sync_info = mybir2.SyncInfo(
                on_wait=keep, on_update=list(si.on_update or [])
            )
    return nc


def _get_program():
    if "nc" not in _CACHE:
        _CACHE["nc"] = _build_program()
    return _CACHE["nc"]


def _host_prep(input, W):
    """Build per-core in_maps. input: [B, I, M]; W: [1, J, I, N, M]."""
    x = np.ascontiguousarray(input, dtype=np.float32)
    W0 = np.ascontiguousarray(W[0], dtype=np.float32)  # [J, I, N, M]

    # mask[(il, m), (il', bl)] = 1 iff il == il'
    il_row = (np.arange(128) // M)[:, None]
    il_col = (np.arange(128) // BL)[None, :]
    mask = (il_row == il_col).astype(np.float32)

    wts = []
    for jg in range(2):
        ws = W0[JL * jg : JL * jg + JL]                 # [JL, I, N, M]
        wts.append(ws.transpose(1, 3, 0, 2).reshape(NCHUNK, 128, JN))
    xcs = []
    for bg in range(4):
        xs = x[BL * bg : BL * bg + BL]                  # [BL, I, M]
        # xc[g, (il, m), bl] = x[bl, 16g+il, m]
        xcs.append(xs.transpose(1, 2, 0).reshape(NCHUNK, 128, BL))

    in_maps = []
    for c in range(NCORES):
        jg, bg = c % 2, c // 2
        in_maps.append(
            {"wb": np.concatenate([wts[jg], xcs[bg]], axis=2), "msk": mask}
        )
    return in_maps


def _host_finish(input, results):
    """Gather selected child capsules and unshard over (j, b)."""
    mask = input.sum(axis=2) != 0.0                     # [B, I]
    keyv = np.where(mask, np.arange(I)[None, :], I)
    sidx = np.sort(keyv, axis=1)[:, :NZC]               # [B, NZC]

    ufull = np.empty((B, I, J, N), dtype=np.float32)
    for c in range(NCORES):
        jg, bg = c % 2, c // 2
        uc = results[c]["u"].reshape(128, NSTRIP, CH_PER_STRIP * JN + PAD)
        uc = uc[:, :, : CH_PER_STRIP * JN].reshape(16, BL, NCHUNK, JL, N)
        # partition p = 8*il + bl; i = 16*chunk + il
        uc = uc.transpose(1, 2, 0, 3, 4).reshape(BL, I, JL, N)
        ufull[BL * bg : BL * bg + BL, :, JL * jg : JL * jg + JL, :] = uc
    sel = ufull[np.arange(B)[:, None], sidx]            # [B, NZC, J, N]
    return np.ascontiguousarray(sel.transpose(0, 2, 1, 3))  # [B, J, NZC, N]


def run_on_cores(input, W, trace=False, **trace_kwargs):
    from concourse.bass_utils import run_bass_kernel_spmd

    nc = _get_program()
    in_maps = _host_prep(input, W)
    res = run_bass_kernel_spmd(
        nc, in_maps, list(range(NCORES)), trace=trace, **trace_kwargs
    )
    return _host_finish(input, res.results), res


def kernel(input, W):
    out, _ = run_on_cores(input, W)
    return out



# revision 15
# speedup vs baseline: 2.2204x; 2.2204x over previous
"""Trainium2 Bass kernel for nn_DenseCapsuleLayer.

Reference computation:
    u_hat[b, j, k, n] = sum_m W[0, j, idx[b,k], n, m] * x[b, idx[b,k], m]
with idx[b, :] the ascending indices of the NZC=1152 non-zero child capsules
of batch b (x is zero elsewhere).

Strategy (8 NeuronCores, 2-way parent-capsule x 4-way batch mesh):
  * Core c owns j in [16*(c%2), 16*(c%2)+16) and b in [8*(c//2), 8*(c//2)+8).
  * Each core computes the DENSE map u_full[b, i, jl, n] for ALL i in bf16
    (x is zero at non-selected i so u_full there is zero and discarded);
    the select/compaction gather over i and the unshard happen on the host.
  * Per 16-wide child-capsule chunk g (i = 16g+il), the PE computes
        out[(il,bl), (jl,n)] = sum_m x[b, 16g+il, m] * W[j, 16g+il, n, m]
    as ONE K=128 bf16 matmul: stationary = [128,128] block-diagonal packing
    of the core's x slice (8 batches), built on device by ONE batched
    broadcast-multiply per strip against a static 0/1 mask; moving = the
    core's W slice pre-transposed to [(il,m), (jl,n)] (256 free columns).
  * Everything over DMA is bf16 (W, x, mask, output) halving HBM bytes;
    PSUM accumulates f32; the PSUM->SBUF copies cast f32->bf16.
  * The CoreSim cost model charges each DMA's transfer to the ISSUING
    engine's queue and queues run concurrently, so the work is spread:
    SP issues the 12 strip in-DMAs; the 72 PSUM->SBUF cast copies are
    split DVE/ACT/Pool (2/2/2 per strip, same pattern every strip so each
    PSUM bank and o_sb column range is always drained by the same
    engine); each strip's output leaves as THREE per-copy-engine region
    DMAs spread over the SP/ACT/Pool queues.  DVE also builds the
    block-diagonal stationaries (one batched mul per strip).

Toolchain constraints: every lowered instruction accepts ONE sync-wait
command, Tile emits a wait per dependency it cannot prove covered, and its
coverage tracking credits only REAL data dependencies.  Each out-DMA reads
exactly one engine's o_sb region, so it naturally carries that single
engine wait.  Per strip and engine, absorber dummies each carry one wait:
adum reads the engine's own previous-strip copy output (self-clock past
all older same-engine hazards), bdum writes the engine's o_sb pad
(carries the region WAR vs the out-DMA three strips back), sdumv (DVE)
reads the fresh W strip, and on PE sdum/bdtdum/gdum read the W strip /
the built stationaries / the previous strip's copy outputs (absorbing the
in-DMA, the stationary RAW, and the PSUM-bank WAR).  All dummy write
targets are per-engine scratch cells written once and never read.  Tile
pool slots are pinned with per-tag buffers so reuse is deterministic.
SP nops park the kernel-tail drain's wait list, and a BIR post-pass drops
DMA waits provably implied by the single wait that is kept.
"""

import numpy as np

B, I, J, M, N = 32, 2304, 32, 8, 16
NZC = I // 2
NCORES = 8
JL = J // 2               # parent capsules per core (16)
JN = JL * N               # 256
BL = B // 4               # batches per core (8)
NCHUNK = I // 16          # 144 chunks of 16 child capsules
NSTRIP = 12
G = NCHUNK // NSTRIP      # 12 chunks per strip
NQUAD = G // 4            # PSUM quads per strip (3)
PADE = 4                  # per-engine o_sb pad columns (bf16)
QJN = 4 * JN              # columns written per quad copy (1024)
REGC = {0: QJN + PADE, 1: 2 * QJN + PADE}   # o_v / o_a widths
SJN = 3 * QJN + 2 * PADE  # 3080 columns per strip in u

V, A, P = "vector", "scalar", "gpsimd"
# copy engine per quad position — SAME every strip (GPSIMD cannot access
# PSUM, so only DVE and ACT can drain the accumulators).
QPAT = [V, A, A]
EIDX = {V: 0, A: 1}

_CACHE = {}


def _ocol(ql):
    """First column of quad ql's copy within its engine's o_sb tile."""
    return 0 if ql == 0 else (ql - 1) * QJN


def _build_program(legalize=True):
    import concourse.bass as bass
    import concourse.mybir as mybir
    import concourse.tile as tile

    f32 = mybir.dt.float32
    bf16 = mybir.dt.bfloat16
    nc = bass.Bass()

    # wb[g, (il,m), 0:256]   = W[j, 16g+il, n, m]  (moving operand)
    # wb[g, (il,m), 256:264] = x[b, 16g+il, m] for the core's 8 batches
    wb = nc.declare_dram_parameter("wb", [NCHUNK, 128, JN + BL], bf16,
                                   isOutput=False)
    # msk[(il,m), (il',bl)] = 1.0 iff il == il'
    msk = nc.declare_dram_parameter("msk", [128, 128], bf16, isOutput=False)
    u = nc.declare_dram_parameter("u", [128, NSTRIP * SJN], bf16,
                                  isOutput=True)

    keep_pref = {}   # DMA inst name -> sem prefix of the wait to keep

    with tile.TileContext(nc, pool_alloc_mode="queue") as tc:
        with (
            tc.tile_pool(name="wpool", bufs=1) as wpool,
            tc.tile_pool(name="bdpool", bufs=1) as bdpool,
            tc.tile_pool(name="opool", bufs=1) as opool,
            tc.tile_pool(name="ppool", bufs=1, space="PSUM") as ppool,
            tc.tile_pool(name="dpool", bufs=1, space="PSUM") as dpool,
            tc.tile_pool(name="zpool", bufs=1) as zpool,
        ):
            ENG = {V: nc.vector, A: nc.scalar, P: nc.gpsimd, "sync": nc.sync}
            last_on = {}          # engine key -> last op (forced order)
            last_eng = {}         # engine key -> last non-DMA engine op

            def emit(key, op):
                prev = last_on.get(key)
                if prev is not None:
                    tile.add_dep_helper(op.ins, prev.ins, sync=False,
                                        reason="ord")
                last_on[key] = op
                if type(op.ins).__name__ != "InstDMACopy":
                    last_eng[key] = op
                return op

            def ecopy(key, dst, src):
                if key == A:
                    return emit(A, nc.scalar.copy(dst, src))
                return emit(key, ENG[key].tensor_copy(dst, src))

            dmy = dpool.tile([1, 160], f32, tag="d")
            z_sb = zpool.tile([128, PADE], bf16, tag="z")
            mask_t = zpool.tile([128, 128], bf16, tag="msk")
            # per-engine scratch: written only by the owning engine, never
            # read -> dummy write targets carry no cross-engine hazards
            scr_v = zpool.tile([1, 32], bf16, tag="scr0")
            scr_a = zpool.tile([1, 32], bf16, tag="scr1")
            scr_p = zpool.tile([1, 32], bf16, tag="scr2")
            scr = {V: scr_v, A: scr_a, P: scr_p}
            scol = {e: [0] for e in (V, A, P)}

            def cell(e):
                c = scol[e][0]
                scol[e][0] += 1
                assert c < 32
                return scr[e][0:1, c: c + 1]

            all_dmas = []

            # --- startup: mask load + per-engine warmups -------------------
            emit(V, nc.vector.memset(z_sb[:, :], 0.0))
            d_msk = emit("sync", nc.sync.dma_start(out=mask_t[:, :],
                                                   in_=msk[:, :]))
            all_dmas.append(d_msk)
            # absorb the mask/z_sb producer ticks on every consumer queue
            emit(V, nc.vector.tensor_copy(cell(V), mask_t[0:1, 0:1]))
            emit(A, nc.scalar.copy(cell(A), z_sb[0:1, 0:1]))
            emit("tensor", nc.tensor.matmul(
                dmy[0:1, 150:151], mask_t[0:32, 0:1], mask_t[0:32, 0:1],
                start=True, stop=True,
            ))

            mask4 = mask_t.rearrange("p (s r c) -> p s r c", s=1, r=16)

            w_tiles = []
            o_tiles = []
            d_out_by = {e: [] for e in (V, A, P)}   # engine -> region DMAs

            def prefetch(s):
                w_sb = wpool.tile([128, G, JN + BL], bf16, tag=f"w{s % 3}")
                w_tiles.append(w_sb)
                d_in = emit("sync", nc.sync.dma_start(
                    out=w_sb[:, :, :],
                    in_=wb[s * G: (s + 1) * G].rearrange("g p c -> p g c"),
                ))
                # post-pass: keep the w-slot WAR (PE readers of strip s-3);
                # the DVE reader and the slot WAW are implied through PE.
                keep_pref[d_in.ins.name] = "PE"
                all_dmas.append(d_in)

            for s in range(3):
                prefetch(s)

            for s in range(NSTRIP):
                w_sb = w_tiles[s]
                o_prev = o_tiles[s - 1] if s >= 1 else None
                # one o_sb tile per copy engine: the pool-slot reuse hazard
                # is whole-tile granular, so a shared tile would tangle both
                # engines' and the out-DMAs' waits together
                o_v = opool.tile([128, REGC[0]], bf16, tag=f"ov{s % 3}")
                o_a = opool.tile([128, REGC[1]], bf16, tag=f"oa{s % 3}")
                o_sb = {V: o_v, A: o_a}
                o_tiles.append(o_sb)

                # adum: each copy engine reads its own previous-strip copy
                # output — the self-sem wait advances the engine's clock so
                # all same-engine WAW/WAR hazards vs older strips are covered
                if s >= 1:
                    for ek in (V, A):
                        ecopy(ek, cell(ek), o_prev[ek][0:1, 0:1])
                # bdum: per-engine pad write; carries the o_sb slot WAR
                # (the engine's region out-DMA of strip s-3 read the pad too)
                ecopy(V, o_v[:, QJN: QJN + PADE], z_sb[:, :])
                ecopy(A, o_a[:, 2 * QJN: 2 * QJN + PADE], z_sb[:, :])

                # absorb the strip in-DMA tick on PE and DVE
                emit("tensor", nc.tensor.matmul(
                    dmy[0:1, s: s + 1], w_sb[0:32, 0, 0:1],
                    w_sb[0:32, 0, 0:1], start=True, stop=True,
                ))
                emit(V, nc.vector.tensor_copy(cell(V),
                                              w_sb[0:1, 0, JN: JN + 1]))

                # build ALL the strip's block-diagonal stationaries at once:
                # bdt[p=(il,m), g, (il',bl)] = x[bl, 16g+il', m] * mask
                bdt = bdpool.tile([128, G, 128], bf16, tag=f"bd{s % 3}")
                x4 = w_sb[:, :, JN: JN + BL].rearrange(
                    "p g (s c) -> p g s c", s=1).broadcast_to([128, G, 16, BL])
                emit(V, nc.vector.tensor_mul(
                    bdt.rearrange("p g (r c) -> p g r c", r=16),
                    x4,
                    mask4.broadcast_to([128, G, 16, BL]),
                ))
                # absorb the stationary-builder tick on PE
                emit("tensor", nc.tensor.matmul(
                    dmy[0:1, 12 + s: 13 + s],
                    bdt[0:32, 0, 0:1], bdt[0:32, 0, 0:1],
                    start=True, stop=True,
                ))

                for ql in range(NQUAD):
                    gq = s * NQUAD + ql
                    ps = ppool.tile([128, 4, JN], f32, tag=f"ps{ql}")
                    if s >= 1:
                        # gdum: read the o_sb region the bank-freeing copy
                        # (same quad position, previous strip) wrote — the
                        # cross-engine wait covers the PSUM-slot WAR
                        opv = o_prev[QPAT[ql]]
                        emit("tensor", nc.tensor.matmul(
                            dmy[0:1, 24 + gq: 25 + gq],
                            opv[0:32, _ocol(ql): _ocol(ql) + 1],
                            opv[0:32, _ocol(ql): _ocol(ql) + 1],
                            start=True, stop=True,
                        ))
                    for h in range(4):
                        gl = 4 * ql + h
                        emit("tensor", nc.tensor.matmul(
                            ps[:, h, :], bdt[:, gl, :], w_sb[:, gl, 0:JN],
                            start=True, stop=True,
                        ))
                    ecopy(QPAT[ql],
                          o_sb[QPAT[ql]][:, _ocol(ql): _ocol(ql) + QJN],
                          ps.rearrange("p a b -> p (a b)"))

                if s + 3 < NSTRIP:
                    prefetch(s + 3)

                # per-engine region out-DMAs on the Pool queue; each reads
                # only one engine's tile so it carries that single wait
                for ek, base, w in ((V, 0, REGC[0]), (A, REGC[0], REGC[1])):
                    d_out = emit(P, nc.gpsimd.dma_start(
                        out=u[:, s * SJN + base: s * SJN + base + w],
                        in_=o_sb[ek][:, :]))
                    keep_pref[d_out.ins.name] = {
                        V: "DVE", A: "Activation"}[ek]
                    all_dmas.append(d_out)
                    d_out_by[ek].append(d_out)

            # tail parking: cover DMAs + engine tails so the kernel-tail
            # drain has at most one wait left
            prev = None
            tails = [op for op in (last_eng.get(k) for k in
                                   ("tensor", V, A)) if op is not None]
            for d in all_dmas + tails:
                w = nc.sync.nop(nofuse=True, hint="park")
                tile.add_dep_helper(w.ins, d.ins, sync=True, reason="park")
                if prev is not None:
                    tile.add_dep_helper(w.ins, prev.ins, sync=False,
                                        reason="ord")
                prev = w

    if not legalize:
        return nc

    # Single-wait legalization: for each multi-wait DMA keep the designated
    # wait — the dropped waits are implied by it through the dummy-op
    # ordering chains (the kept tick is only reached after the dropped
    # dependencies completed).
    import concourse.mybir as mybir2

    for blk in nc.m.functions[0].blocks:
        for inst in blk.instructions:
            si = inst.sync_info
            if si is None or not si.on_wait or len(si.on_wait) < 2:
                continue
            if type(inst).__name__ != "InstDMACopy":
                dbg = inst.debug
                raise RuntimeError(
                    f"unexpected multi-wait {inst.name} "
                    f"({type(inst).__name__} engine={inst.engine} "
                    f"line={getattr(dbg, 'lineno', None)}) "
                    f"{[w.ant_name for w in si.on_wait]}"
                )
            pref = keep_pref.get(inst.name)
            if pref is None:
                raise RuntimeError(
                    f"multi-wait DMA {inst.name} with no keep rule: "
                    f"{[w.ant_name for w in si.on_wait]}"
                )
            keep = [w for w in si.on_wait if w.ant_name.startswith(pref)]
            if len(keep) != 1:
                raise RuntimeError(
                    f"{inst.name}: expected one {pref} wait, got "
                    f"{[w.ant_name for w in si.on_wait]}"
                )
            inst.sync_info = mybir2.SyncInfo(
                on_wait=keep, on_update=list(si.on_update or [])
            )
    return nc


def _get_program():
    if "nc" not in _CACHE:
        _CACHE["nc"] = _build_program()
    return _CACHE["nc"]


def _bf16(a):
    import ml_dtypes
    return np.asarray(a, dtype=np.float32).astype(ml_dtypes.bfloat16)


def _host_prep(input, W):
    """Build per-core in_maps. input: [B, I, M]; W: [1, J, I, N, M]."""
    x = np.ascontiguousarray(input, dtype=np.float32)
    W0 = np.ascontiguousarray(W[0], dtype=np.float32)  # [J, I, N, M]

    # mask[(il, m), (il', bl)] = 1 iff il == il'
    il_row = (np.arange(128) // M)[:, None]
    il_col = (np.arange(128) // BL)[None, :]
    mask = _bf16((il_row == il_col).astype(np.float32))

    wts = []
    for jg in range(2):
        ws = W0[JL * jg: JL * jg + JL]                  # [JL, I, N, M]
        wts.append(_bf16(ws.transpose(1, 3, 0, 2).reshape(NCHUNK, 128, JN)))
    xcs = []
    for bg in range(4):
        xs = x[BL * bg: BL * bg + BL]                   # [BL, I, M]
        # xc[g, (il, m), bl] = x[bl, 16g+il, m]
        xcs.append(_bf16(xs.transpose(1, 2, 0).reshape(NCHUNK, 128, BL)))

    in_maps = []
    for c in range(NCORES):
        jg, bg = c % 2, c // 2
        in_maps.append(
            {"wb": np.concatenate([wts[jg], xcs[bg]], axis=2), "msk": mask}
        )
    return in_maps


def _host_finish(input, results):
    """Gather selected child capsules and unshard over (j, b)."""
    mask = input.sum(axis=2) != 0.0                     # [B, I]
    keyv = np.where(mask, np.arange(I)[None, :], I)
    sidx = np.sort(keyv, axis=1)[:, :NZC]               # [B, NZC]

    ufull = np.empty((B, I, J, N), dtype=np.float32)
    for c in range(NCORES):
        jg, bg = c % 2, c // 2
        uc = np.asarray(results[c]["u"], dtype=np.float32)
        uc = uc.reshape(128, NSTRIP, SJN)
        uc = np.concatenate(
            [uc[:, :, :QJN], uc[:, :, QJN + PADE: 3 * QJN + PADE]], axis=2)
        uc = uc.reshape(16, BL, NCHUNK, JL, N)
        # partition p = 8*il + bl; i = 16*chunk + il
        uc = uc.transpose(1, 2, 0, 3, 4).reshape(BL, I, JL, N)
        ufull[BL * bg: BL * bg + BL, :, JL * jg: JL * jg + JL, :] = uc
    sel = ufull[np.arange(B)[:, None], sidx]            # [B, NZC, J, N]
    return np.ascontiguousarray(sel.transpose(0, 2, 1, 3))  # [B, J, NZC, N]


def run_on_cores(input, W, trace=False, **trace_kwargs):
    from concourse.bass_utils import run_bass_kernel_spmd

    nc = _get_program()
    in_maps = _host_prep(input, W)
    res = run_bass_kernel_spmd(
        nc, in_maps, list(range(NCORES)), trace=trace, **trace_kwargs
    )
    return _host_finish(input, res.results), res


def kernel(input, W):
    out, _ = run_on_cores(input, W)
    return out


if __name__ == "__main__":
    nc = _get_program()
    n_inst = sum(len(b.instructions) for b in nc.m.functions[0].blocks)
    print(f"built OK: {n_inst} instructions")
    from concourse.bass_interp import CoreSim
    import reference as R
    import jax
    with jax.default_device(jax.devices("cpu")[0]):
        inputs = {k: np.asarray(v) for k, v in R.setup_inputs().items()}
    sim = CoreSim(nc)
    sim.assign_tensors(_host_prep(inputs["input"], inputs["W"])[0])
    sim.simulate()
    print(f"sim time: {sim.time} ns")


# revision 35
# speedup vs baseline: 2.5674x; 1.1562x over previous
"""Trainium2 Bass kernel for nn_DenseCapsuleLayer.

Reference computation:
    u_hat[b, j, k, n] = sum_m W[0, j, idx[b,k], n, m] * x[b, idx[b,k], m]
with idx[b, :] the ascending indices of the NZC=1152 non-zero child capsules
of batch b (x is zero elsewhere).

Strategy (8 NeuronCores, 2-way parent-capsule x 4-way batch mesh):
  * Core c owns j in [16*(c%2), 16*(c%2)+16) and b in [8*(c//2), 8*(c//2)+8).
  * Each core computes the DENSE map u_full[b, i, jl, n] for ALL i in bf16
    (x is zero at non-selected i so u_full there is zero and discarded);
    the select/compaction gather over i and the unshard happen on the host.
  * Per 16-wide child-capsule chunk g (i = 16g+il), the PE computes
        out[(il,bl), (jl,n)] = sum_m x[b, 16g+il, m] * W[j, 16g+il, n, m]
    as ONE K=128 bf16 matmul: stationary = [128,128] block-diagonal packing
    of the core's x slice (8 batches), built on device by ONE batched
    broadcast-multiply per strip against a static 0/1 mask (built one strip
    AHEAD so the PE is never gated on the builder); moving = the core's W
    slice pre-transposed to [(il,m), (jl,n)] (256 free columns).
  * Everything over DMA is bf16 (W, x, mask, output) halving HBM bytes;
    PSUM accumulates f32; the PSUM->SBUF copies cast f32->bf16.
  * The CoreSim cost model charges each DMA's transfer to the ISSUING
    engine's queue and queues run concurrently, so the work is spread:
    SP issues most strip in-DMAs (the first three are split across
    SP/ACT/Pool so the pipeline fills fast); the 36 PSUM->SBUF cast
    copies are split DVE/ACT (1/2 per strip — GPSIMD cannot touch PSUM);
    out-DMAs leave per copy-engine region, mostly on the Pool queue.
  * The output staging tiles (o_v/o_a) and the stationaries (bdt) are NOT
    pool-recycled: each strip gets its own SBUF tile, which deletes every
    slot-reuse WAR/WAW hazard (and its dummy-op tax) on the copy engines.
    Only the W stream (3 slots) and PSUM (4 quad slots) recycle.

Toolchain constraints: every lowered instruction accepts ONE sync-wait
command, Tile emits a wait per dependency it cannot prove covered, and its
coverage tracking credits only REAL data dependencies.  Per strip, dummy
ops each carry one wait: sdumv (DVE) reads the fresh W strip so the
stationary-builder needs no DMA wait, and on PE zero-cost bare ldweights
reads absorb the in-DMA (sdum), the stationary RAW (bdtdum), and the
PSUM-slot WAR (gdum reads the o region the bank-freeing copy wrote).
Matmuls may carry a PE self-wait (PSUM slot WAW), copies carry their PE
RAW, out-DMAs carry their copy engine's tick.  SP nops park the
kernel-tail drain's wait list, and a BIR post-pass drops the in-DMA waits
(W-slot WAR) that are provably implied by the kept PE wait.
"""

import numpy as np

B, I, J, M, N = 32, 2304, 32, 8, 16
NZC = I // 2
NCORES = 8
JL = J // 2               # parent capsules per core (16)
JN = JL * N               # 256
BL = B // 4               # batches per core (8)
NCHUNK = I // 16          # 144 chunks of 16 child capsules
NSTRIP = 18
G = NCHUNK // NSTRIP      # 8 chunks per strip
GV = 2                    # chunks drained by DVE per strip (1 PSUM bank)
GA = G - GV               # chunks drained by ACT per strip (6 = 3 banks)
VJN = GV * JN             # 768 columns in the DVE region
AJN = GA * JN             # 1280 columns in the ACT region
SJN = G * JN              # 2048 columns per strip in u

V, A, P = "vector", "scalar", "gpsimd"

_CACHE = {}


def _build_program(legalize=True):
    import concourse.bass as bass
    import concourse.mybir as mybir
    import concourse.tile as tile

    f32 = mybir.dt.float32
    bf16 = mybir.dt.bfloat16
    nc = bass.Bass()

    # wb[g, (il,m), 0:256] = W[j, 16g+il, n, m]  (moving operand)
    wb = nc.declare_dram_parameter("wb", [NCHUNK, 128, JN], bf16,
                                   isOutput=False)
    # xb[(il,m), g, bl] = x[bl, 16g+il, m] for the core's 8 batches
    xb = nc.declare_dram_parameter("xb", [128, NCHUNK, BL], bf16,
                                   isOutput=False)
    # msk[(il,m), (il',bl)] = 1.0 iff il == il'
    msk = nc.declare_dram_parameter("msk", [128, 128], bf16, isOutput=False)
    u = nc.declare_dram_parameter("u", [128, NSTRIP * SJN], bf16,
                                  isOutput=True)

    keep_pref = {}   # DMA inst name -> sem prefix of the wait to keep

    with tile.TileContext(nc, pool_alloc_mode="queue") as tc:
        with (
            tc.tile_pool(name="wpool", bufs=1) as wpool,
            tc.tile_pool(name="ppool", bufs=1, space="PSUM") as ppool,
            tc.tile_pool(name="zpool", bufs=1) as zpool,
        ):
            ENG = {V: nc.vector, A: nc.scalar, P: nc.gpsimd, "sync": nc.sync}
            last_on = {}          # engine key -> last op (forced order)
            last_eng = {}         # engine key -> last non-DMA engine op

            def emit(key, op):
                prev = last_on.get(key)
                if prev is not None:
                    tile.add_dep_helper(op.ins, prev.ins, sync=False,
                                        reason="ord")
                last_on[key] = op
                if type(op.ins).__name__ != "InstDMACopy":
                    last_eng[key] = op
                return op

            def ecopy(key, dst, src):
                if key == A:
                    return emit(A, nc.scalar.copy(dst, src))
                return emit(key, ENG[key].tensor_copy(dst, src))

            mask_t = zpool.tile([128, 128], bf16, tag="msk")
            scr_v = zpool.tile([1, 32], bf16, tag="scr0")
            scr_p = zpool.tile([1, 32], bf16, tag="scr1")
            scr_a = zpool.tile([1, 8], bf16, tag="scr2")
            pcol = [0]

            def pcell():
                c = pcol[0]
                pcol[0] += 1
                assert c < 32
                return scr_p[0:1, c: c + 1]
            scol = [0]

            def vcell():
                c = scol[0]
                scol[0] += 1
                assert c < 32
                return scr_v[0:1, c: c + 1]

            x_sb = zpool.tile([128, NCHUNK, BL], bf16, tag="x")
            # per-strip unpooled tiles (no slot reuse -> no WAR/WAW tax);
            # one o tile per strip: DVE writes cols [0:VJN), ACT the rest,
            # and a single out-DMA moves the whole strip
            o_ts, bdts = [], []
            for s in range(NSTRIP):
                t1 = zpool.tile([128, SJN], bf16, tag=f"o{s}")
                t3 = zpool.tile([128, G, 128], bf16, tag=f"bd{s}")
                o_ts.append(t1)
                bdts.append(t3)

            all_dmas = []

            # --- startup ---------------------------------------------------
            d_msk = emit("sync", nc.sync.dma_start(out=mask_t[:, :],
                                                   in_=msk[:, :]))
            all_dmas.append(d_msk)
            # x ships once, early, on the ACT queue
            d_x = emit(A, nc.scalar.dma_start(out=x_sb[:, :, :],
                                              in_=xb[:, :, :]))
            all_dmas.append(d_x)
            # absorb the mask and x ticks on DVE (the only consumers)
            emit(V, nc.vector.tensor_copy(vcell(), mask_t[0:1, 0:1]))
            emit(V, nc.vector.tensor_copy(vcell(), x_sb[0:1, 0, 0:1]))

            mask4 = mask_t.rearrange("p (s r c) -> p s r c", s=1, r=16)

            w_tiles = []

            def prefetch(s, qk="sync", halves=False):
                w_sb = wpool.tile([128, G, JN], bf16, tag=f"w{s % 5}")
                w_tiles.append(w_sb)
                if halves:
                    h = G // 2
                    for qq, lo in (("sync", 0), (A, h)):
                        d_in = emit(qq, ENG[qq].dma_start(
                            out=w_sb[:, lo: lo + h, :],
                            in_=wb[s * G + lo: s * G + lo + h].rearrange(
                                "g p c -> p g c"),
                        ))
                        keep_pref[d_in.ins.name] = "PE"
                        all_dmas.append(d_in)
                else:
                    d_in = emit(qk, ENG[qk].dma_start(
                        out=w_sb[:, :, :],
                        in_=wb[s * G: (s + 1) * G].rearrange("g p c -> p g c"),
                    ))
                    keep_pref[d_in.ins.name] = "PE"
                    all_dmas.append(d_in)

            def build_bdt(s):
                """Stationary build for strip s (reads x_sb + mask)."""
                x4 = x_sb[:, s * G: (s + 1) * G, :].rearrange(
                    "p g (s c) -> p g s c", s=1).broadcast_to([128, G, 16, BL])
                emit(V, nc.vector.tensor_mul(
                    bdts[s].rearrange("p g (r c) -> p g r c", r=16),
                    x4,
                    mask4.broadcast_to([128, G, 16, BL]),
                ))

            # split the first strip's load across two queues so the pipeline
            # fills fast; spread the next two over otherwise-idle queues
            prefetch(0, halves=True)
            prefetch(1, "sync")
            prefetch(2, "sync")
            prefetch(3, "sync")
            # ACT warmup: pays the activation-table load during startup
            # idle, after the startup DMAs ACT issues
            emit(A, nc.scalar.copy(scr_a[0:1, 0:1], x_sb[0:1, 0, 0:1]))
            build_bdt(0)

            for s in range(NSTRIP):
                w_sb = w_tiles[s]
                o_t, bdt = o_ts[s], bdts[s]

                # absorb the strip in-DMA tick on PE, then the stationary
                # builder's tick (both zero-cost bare weight loads)
                emit("tensor", nc.tensor.ldweights(w_sb[0:32, 0, 0:1]))
                emit("tensor", nc.tensor.ldweights(bdt[0:32, 0, 0:1]))

                # build the NEXT strip's stationaries before this strip's
                # DVE copy so the PE is never gated on the builder
                if s + 1 < NSTRIP:
                    build_bdt(s + 1)

                acopies = []
                for ek, base, gn in ((A, VJN, GA), (V, 0, GV)):
                    tagc = "A" if ek == V else "B"
                    ps = ppool.tile([128, gn, JN], f32,
                                    tag=f"ps{tagc}{s % 2}")
                    if s >= 2:
                        # gdum: read the o region the bank-freeing copy (same
                        # slot, two strips back) wrote — the cross-engine
                        # wait covers the PSUM-slot WAR
                        emit("tensor", nc.tensor.ldweights(
                            o_ts[s - 2][0:32, base: base + 1]))
                    for h in range(gn):
                        gl = (0 if ek == V else GV) + h
                        emit("tensor", nc.tensor.matmul(
                            ps[:, h, :], bdt[:, gl, :], w_sb[:, gl, 0:JN],
                            start=True, stop=True,
                        ))
                    if ek == V or s < NSTRIP - 1:
                        cp = ecopy(ek, o_t[:, base: base + gn * JN],
                                   ps.rearrange("p a b -> p (a b)"))
                        if ek == A:
                            acopies.append((cp, base, gn * JN))
                    else:
                        # last strip: three small ACT copies so the final
                        # copy->out chain is short
                        for j in range(3):
                            cp = ecopy(A, o_t[:, base + 2 * j * JN:
                                              base + 2 * (j + 1) * JN],
                                       ps[:, 2 * j: 2 * j + 2, :].rearrange(
                                           "p a b -> p (a b)"))
                            acopies.append((cp, base + 2 * j * JN, 2 * JN))

                if s + 4 < NSTRIP:
                    prefetch(s + 4)

                # pdum: near-free Pool op reading the DVE region — its DVE
                # wait lets the strip out-DMA(s) carry only the ACT wait
                emit(P, nc.gpsimd.tensor_copy(pcell(), o_t[0:1, 0:1]))
                if s < NSTRIP - 1:
                    d_out = emit(P, nc.gpsimd.dma_start(
                        out=u[:, s * SJN: (s + 1) * SJN], in_=o_t[:, :]))
                    keep_pref[d_out.ins.name] = "Activation"
                    all_dmas.append(d_out)
                else:
                    # last strip: out-DMA per ACT copy (the first also
                    # carries the DVE region, covered by the pdum)
                    for j, (cp, cb, cw) in enumerate(acopies):
                        lo = 0 if j == 0 else cb
                        hi = cb + cw
                        d_out = emit(P, nc.gpsimd.dma_start(
                            out=u[:, s * SJN + lo: s * SJN + hi],
                            in_=o_t[:, lo: hi]))
                        keep_pref[d_out.ins.name] = "Activation"
                        all_dmas.append(d_out)

            # tail parking: cover DMAs + engine tails so the kernel-tail
            # drain has at most one wait left.  SWDGE (Pool) DMA sems only
            # have race-free wait values at their FINAL cumulative count, so
            # park just the last Pool DMA per DMASW lane.
            pool_dmas = [d for d in all_dmas
                         if d.ins.engine == ENG[P].engine]
            park_pool = set()
            for lane in range(8):
                lst = pool_dmas[lane::8]
                if lst:
                    park_pool.add(id(lst[-1]))
            parked = [d for d in all_dmas
                      if d.ins.engine != ENG[P].engine
                      or id(d) in park_pool]
            prev = None
            tails = [op for op in (last_eng.get(k) for k in
                                   ("tensor", V, A)) if op is not None]
            for d in parked + tails:
                w = nc.sync.nop(nofuse=True, hint="park")
                tile.add_dep_helper(w.ins, d.ins, sync=True, reason="park")
                if prev is not None:
                    tile.add_dep_helper(w.ins, prev.ins, sync=False,
                                        reason="ord")
                prev = w

    if not legalize:
        return nc

    # Single-wait legalization: for each multi-wait DMA keep the designated
    # wait — the dropped waits are implied by it through the dummy-op
    # ordering chains (the kept tick is only reached after the dropped
    # dependencies completed).
    import concourse.mybir as mybir2

    for blk in nc.m.functions[0].blocks:
        for inst in blk.instructions:
            si = inst.sync_info
            if si is None or not si.on_wait or len(si.on_wait) < 2:
                continue
            if type(inst).__name__ != "InstDMACopy":
                dbg = inst.debug
                raise RuntimeError(
                    f"unexpected multi-wait {inst.name} "
                    f"({type(inst).__name__} engine={inst.engine} "
                    f"line={getattr(dbg, 'lineno', None)}) "
                    f"{[w.ant_name for w in si.on_wait]}"
                )
            pref = keep_pref.get(inst.name)
            if pref is None:
                raise RuntimeError(
                    f"multi-wait DMA {inst.name} with no keep rule: "
                    f"{[w.ant_name for w in si.on_wait]}"
                )
            keep = [w for w in si.on_wait if w.ant_name.startswith(pref)]
            if len(keep) != 1:
                raise RuntimeError(
                    f"{inst.name}: expected one {pref} wait, got "
                    f"{[w.ant_name for w in si.on_wait]}"
                )
            inst.sync_info = mybir2.SyncInfo(
                on_wait=keep, on_update=list(si.on_update or [])
            )
    return nc


def _get_program():
    if "nc" not in _CACHE:
        _CACHE["nc"] = _build_program()
    return _CACHE["nc"]


def _bf16(a):
    import ml_dtypes
    return np.asarray(a, dtype=np.float32).astype(ml_dtypes.bfloat16)


def _host_prep(input, W):
    """Build per-core in_maps. input: [B, I, M]; W: [1, J, I, N, M]."""
    x = np.ascontiguousarray(input, dtype=np.float32)
    W0 = np.ascontiguousarray(W[0], dtype=np.float32)  # [J, I, N, M]

    # mask[(il, m), (il', bl)] = 1 iff il == il'
    il_row = (np.arange(128) // M)[:, None]
    il_col = (np.arange(128) // BL)[None, :]
    mask = _bf16((il_row == il_col).astype(np.float32))

    wts = []
    for jg in range(2):
        ws = W0[JL * jg: JL * jg + JL]                  # [JL, I, N, M]
        wts.append(_bf16(ws.transpose(1, 3, 0, 2).reshape(NCHUNK, 128, JN)))
    xcs = []
    for bg in range(4):
        xs = x[BL * bg: BL * bg + BL]                   # [BL, I, M]
        # xc[g, (il, m), bl] = x[bl, 16g+il, m]
        xcs.append(_bf16(xs.transpose(1, 2, 0).reshape(NCHUNK, 128, BL)))

    in_maps = []
    for c in range(NCORES):
        jg, bg = c % 2, c // 2
        in_maps.append(
            {"wb": wts[jg],
             "xb": np.ascontiguousarray(xcs[bg].transpose(1, 0, 2)),
             "msk": mask}
        )
    return in_maps


def _host_finish(input, results):
    """Gather selected child capsules and unshard over (j, b)."""
    mask = input.sum(axis=2) != 0.0                     # [B, I]
    keyv = np.where(mask, np.arange(I)[None, :], I)
    sidx = np.sort(keyv, axis=1)[:, :NZC]               # [B, NZC]

    ufull = np.empty((B, I, J, N), dtype=np.float32)
    for c in range(NCORES):
        jg, bg = c % 2, c // 2
        uc = np.asarray(results[c]["u"], dtype=np.float32)
        # u columns are chunk-major: [strip, chunk-in-strip, jl, n]
        uc = uc.reshape(16, BL, NCHUNK, JL, N)
        # partition p = 8*il + bl; i = 16*chunk + il
        uc = uc.transpose(1, 2, 0, 3, 4).reshape(BL, I, JL, N)
        ufull[BL * bg: BL * bg + BL, :, JL * jg: JL * jg + JL, :] = uc
    sel = ufull[np.arange(B)[:, None], sidx]            # [B, NZC, J, N]
    return np.ascontiguousarray(sel.transpose(0, 2, 1, 3))  # [B, J, NZC, N]


def run_on_cores(input, W, trace=False, **trace_kwargs):
    from concourse.bass_utils import run_bass_kernel_spmd

    nc = _get_program()
    in_maps = _host_prep(input, W)
    res = run_bass_kernel_spmd(
        nc, in_maps, list(range(NCORES)), trace=trace, **trace_kwargs
    )
    return _host_finish(input, res.results), res


def kernel(input, W):
    out, _ = run_on_cores(input, W)
    return out


if __name__ == "__main__":
    nc = _get_program()
    n_inst = sum(len(b.instructions) for b in nc.m.functions[0].blocks)
    print(f"built OK: {n_inst} instructions")
    from concourse.bass_interp import CoreSim
    import reference as R
    import jax
    with jax.default_device(jax.devices("cpu")[0]):
        inputs = {k: np.asarray(v) for k, v in R.setup_inputs().items()}
    sim = CoreSim(nc)
    sim.assign_tensors(_host_prep(inputs["input"], inputs["W"])[0])
    sim.simulate()
    print(f"sim time: {sim.time} ns")


# revision 39
# speedup vs baseline: 2.7489x; 1.0707x over previous
"""Trainium2 Bass kernel for nn_DenseCapsuleLayer.

Reference computation:
    u_hat[b, j, k, n] = sum_m W[0, j, idx[b,k], n, m] * x[b, idx[b,k], m]
with idx[b, :] the ascending indices of the NZC=1152 non-zero child capsules
of batch b (x is zero elsewhere).

Strategy (8 NeuronCores, 2-way parent-capsule x 4-way batch mesh):
  * Core c owns j in [16*(c%2), 16*(c%2)+16) and b in [8*(c//2), 8*(c//2)+8).
  * Each core computes the DENSE map u_full[b, i, jl, n] for ALL i in bf16
    (x is zero at non-selected i so u_full there is zero and discarded);
    the select/compaction gather over i and the unshard happen on the host.
  * Per 16-wide child-capsule chunk g (i = 16g+il), the PE computes
        out[(il,bl), (jl,n)] = sum_m x[b, 16g+il, m] * W[j, 16g+il, n, m]
    as ONE K=128 bf16 matmul: stationary = [128,128] block-diagonal packing
    of the core's x slice (8 batches), built on device by ONE batched
    broadcast-multiply per strip against a static 0/1 mask (built one strip
    AHEAD so the PE is never gated on the builder); moving = the core's W
    slice pre-transposed to [(il,m), (jl,n)] (256 free columns).
  * Everything over DMA is bf16 (W, x, mask, output) halving HBM bytes;
    PSUM accumulates f32; the PSUM->SBUF copies cast f32->bf16.
  * The CoreSim cost model charges each DMA's transfer to the ISSUING
    engine's queue and queues run concurrently, so the work is spread:
    SP issues most strip in-DMAs (the first three are split across
    SP/ACT/Pool so the pipeline fills fast); the 36 PSUM->SBUF cast
    copies are split DVE/ACT (1/2 per strip — GPSIMD cannot touch PSUM);
    out-DMAs leave per copy-engine region, mostly on the Pool queue.
  * The output staging tiles (o_v/o_a) and the stationaries (bdt) are NOT
    pool-recycled: each strip gets its own SBUF tile, which deletes every
    slot-reuse WAR/WAW hazard (and its dummy-op tax) on the copy engines.
    Only the W stream (3 slots) and PSUM (4 quad slots) recycle.

Toolchain constraints: every lowered instruction accepts ONE sync-wait
command, Tile emits a wait per dependency it cannot prove covered, and its
coverage tracking credits only REAL data dependencies.  Per strip, dummy
ops each carry one wait: sdumv (DVE) reads the fresh W strip so the
stationary-builder needs no DMA wait, and on PE zero-cost bare ldweights
reads absorb the in-DMA (sdum), the stationary RAW (bdtdum), and the
PSUM-slot WAR (gdum reads the o region the bank-freeing copy wrote).
Matmuls may carry a PE self-wait (PSUM slot WAW), copies carry their PE
RAW, out-DMAs carry their copy engine's tick.  SP nops park the
kernel-tail drain's wait list, and a BIR post-pass drops the in-DMA waits
(W-slot WAR) that are provably implied by the kept PE wait.
"""

import numpy as np

B, I, J, M, N = 32, 2304, 32, 8, 16
NZC = I // 2
NCORES = 8
JL = J // 2               # parent capsules per core (16)
JN = JL * N               # 256
BL = B // 4               # batches per core (8)
NCHUNK = I // 16          # 144 chunks of 16 child capsules
NSTRIP = 18
G = NCHUNK // NSTRIP      # 8 chunks per strip
# DVE/ACT copy split alternates (2,6)/(4,4) by strip parity: PSUM tags
# psA0[2]+psA1[4]+psB0[6]+psB1[4] = exactly 8 banks, and the average
# balances DVE (which also builds stationaries) against ACT.
GVP = [4, 2]              # chunks drained by DVE, by strip parity
SJN = G * JN              # 2048 columns per strip in u

V, A, P = "vector", "scalar", "gpsimd"

_CACHE = {}


def _build_program(legalize=True):
    import concourse.bass as bass
    import concourse.mybir as mybir
    import concourse.tile as tile

    f32 = mybir.dt.float32
    bf16 = mybir.dt.bfloat16
    nc = bass.Bass()

    # wb[g, (il,m), 0:256] = W[j, 16g+il, n, m]  (moving operand)
    wb = nc.declare_dram_parameter("wb", [NCHUNK, 128, JN], bf16,
                                   isOutput=False)
    # xb[(il,m), g, bl] = x[bl, 16g+il, m] for the core's 8 batches
    xb = nc.declare_dram_parameter("xb", [128, NCHUNK, BL], bf16,
                                   isOutput=False)
    # msk[(il,m), (il',bl)] = 1.0 iff il == il'
    msk = nc.declare_dram_parameter("msk", [128, 128], bf16, isOutput=False)
    u = nc.declare_dram_parameter("u", [128, NSTRIP * SJN], bf16,
                                  isOutput=True)

    keep_pref = {}   # DMA inst name -> sem prefix of the wait to keep

    with tile.TileContext(nc, pool_alloc_mode="queue") as tc:
        with (
            tc.tile_pool(name="wpool", bufs=1) as wpool,
            tc.tile_pool(name="ppool", bufs=1, space="PSUM") as ppool,
            tc.tile_pool(name="zpool", bufs=1) as zpool,
        ):
            ENG = {V: nc.vector, A: nc.scalar, P: nc.gpsimd, "sync": nc.sync}
            last_on = {}          # engine key -> last op (forced order)
            last_eng = {}         # engine key -> last non-DMA engine op

            def emit(key, op):
                prev = last_on.get(key)
                if prev is not None:
                    tile.add_dep_helper(op.ins, prev.ins, sync=False,
                                        reason="ord")
                last_on[key] = op
                if type(op.ins).__name__ != "InstDMACopy":
                    last_eng[key] = op
                return op

            def ecopy(key, dst, src):
                if key == A:
                    return emit(A, nc.scalar.copy(dst, src))
                return emit(key, ENG[key].tensor_copy(dst, src))

            mask_t = zpool.tile([128, 128], bf16, tag="msk")
            scr_v = zpool.tile([1, 32], bf16, tag="scr0")
            scr_p = zpool.tile([1, 32], bf16, tag="scr1")
            scr_a = zpool.tile([1, 8], bf16, tag="scr2")
            pcol = [0]

            def pcell():
                c = pcol[0]
                pcol[0] += 1
                assert c < 32
                return scr_p[0:1, c: c + 1]
            scol = [0]

            def vcell():
                c = scol[0]
                scol[0] += 1
                assert c < 32
                return scr_v[0:1, c: c + 1]

            x_sb = zpool.tile([128, NCHUNK, BL], bf16, tag="x")
            # per-strip unpooled tiles (no slot reuse -> no WAR/WAW tax);
            # one o tile per strip: DVE writes cols [0:VJN), ACT the rest,
            # and a single out-DMA moves the whole strip
            o_ts, bdts = [], []
            for s in range(NSTRIP):
                t1 = zpool.tile([128, SJN], bf16, tag=f"o{s}")
                t3 = zpool.tile([128, G, 128], bf16, tag=f"bd{s}")
                o_ts.append(t1)
                bdts.append(t3)

            all_dmas = []

            # --- startup ---------------------------------------------------
            d_msk = emit(A, nc.scalar.dma_start(out=mask_t[:, :],
                                                in_=msk[:, :]))
            all_dmas.append(d_msk)
            # x ships once, early, on the ACT queue
            d_x = emit(A, nc.scalar.dma_start(out=x_sb[:, :, :],
                                              in_=xb[:, :, :]))
            all_dmas.append(d_x)
            # absorb the mask and x ticks on DVE (the only consumers)
            emit(V, nc.vector.tensor_copy(vcell(), mask_t[0:1, 0:1]))
            emit(V, nc.vector.tensor_copy(vcell(), x_sb[0:1, 0, 0:1]))

            mask4 = mask_t.rearrange("p (s r c) -> p s r c", s=1, r=16)

            w_tiles = []

            def prefetch(s, qk="sync", halves=False):
                w_sb = wpool.tile([128, G, JN], bf16, tag=f"w{s % 5}")
                w_tiles.append(w_sb)
                if halves:
                    h = G // 2
                    for qq, lo in (("sync", 0), (A, h)):
                        d_in = emit(qq, ENG[qq].dma_start(
                            out=w_sb[:, lo: lo + h, :],
                            in_=wb[s * G + lo: s * G + lo + h].rearrange(
                                "g p c -> p g c"),
                        ))
                        keep_pref[d_in.ins.name] = "PE"
                        all_dmas.append(d_in)
                else:
                    d_in = emit(qk, ENG[qk].dma_start(
                        out=w_sb[:, :, :],
                        in_=wb[s * G: (s + 1) * G].rearrange("g p c -> p g c"),
                    ))
                    keep_pref[d_in.ins.name] = "PE"
                    all_dmas.append(d_in)

            def build_bdt(s):
                """Stationary build for strip s (reads x_sb + mask)."""
                x4 = x_sb[:, s * G: (s + 1) * G, :].rearrange(
                    "p g (s c) -> p g s c", s=1).broadcast_to([128, G, 16, BL])
                emit(V, nc.vector.tensor_mul(
                    bdts[s].rearrange("p g (r c) -> p g r c", r=16),
                    x4,
                    mask4.broadcast_to([128, G, 16, BL]),
                ))

            # split the first strip's load across two queues so the pipeline
            # fills fast; spread the next two over otherwise-idle queues
            prefetch(0, halves=True)
            prefetch(1, "sync")
            prefetch(2, "sync")
            prefetch(3, "sync")
            # ACT warmup: pays the activation-table load during startup
            # idle, after the startup DMAs ACT issues
            emit(A, nc.scalar.copy(scr_a[0:1, 0:1], x_sb[0:1, 0, 0:1]))
            build_bdt(0)

            for s in range(NSTRIP):
                w_sb = w_tiles[s]
                o_t, bdt = o_ts[s], bdts[s]

                # absorb the strip in-DMA tick on PE, then the stationary
                # builder's tick (both zero-cost bare weight loads)
                emit("tensor", nc.tensor.ldweights(w_sb[0:32, 0, 0:1]))
                emit("tensor", nc.tensor.ldweights(bdt[0:32, 0, 0:1]))

                # build the NEXT strip's stationaries before this strip's
                # DVE copy so the PE is never gated on the builder
                if s + 1 < NSTRIP:
                    build_bdt(s + 1)

                acopies = []
                gv = GVP[s % 2]
                vjn = gv * JN
                for ek, base, gn in ((A, vjn, G - gv), (V, 0, gv)):
                    tagc = "A" if ek == V else "B"
                    ps = ppool.tile([128, gn, JN], f32,
                                    tag=f"ps{tagc}{s % 2}")
                    assert gn == (GVP[s % 2] if ek == V else G - GVP[s % 2])
                    if s >= 2:
                        # gdum: read the o region the bank-freeing copy (same
                        # slot, two strips back) wrote — the cross-engine
                        # wait covers the PSUM-slot WAR
                        emit("tensor", nc.tensor.ldweights(
                            o_ts[s - 2][0:32, base: base + 1]))
                    for h in range(gn):
                        gl = (0 if ek == V else gv) + h
                        emit("tensor", nc.tensor.matmul(
                            ps[:, h, :], bdt[:, gl, :], w_sb[:, gl, 0:JN],
                            start=True, stop=True,
                        ))
                    if ek == V or s < NSTRIP - 1:
                        cp = ecopy(ek, o_t[:, base: base + gn * JN],
                                   ps.rearrange("p a b -> p (a b)"))
                        if ek == A:
                            acopies.append((cp, base, gn * JN))
                    else:
                        # last strip: small 2-chunk ACT copies so the final
                        # copy->out chain is short
                        for j in range(gn // 2):
                            cp = ecopy(A, o_t[:, base + 2 * j * JN:
                                              base + 2 * (j + 1) * JN],
                                       ps[:, 2 * j: 2 * j + 2, :].rearrange(
                                           "p a b -> p (a b)"))
                            acopies.append((cp, base + 2 * j * JN, 2 * JN))

                if s + 4 < NSTRIP:
                    prefetch(s + 4)

                # pdum: near-free Pool op reading the DVE region — its DVE
                # wait lets the strip out-DMA(s) carry only the ACT wait
                emit(P, nc.gpsimd.tensor_copy(pcell(), o_t[0:1, 0:1]))
                if s in (NSTRIP - 3, NSTRIP - 2):
                    # late strips go out on SP (idle by then) as separate
                    # V/A-region DMAs so each carries one engine wait
                    for base, w, pref in ((0, vjn, "DVE"),
                                          (vjn, SJN - vjn, "Activation")):
                        d_out = emit("sync", nc.sync.dma_start(
                            out=u[:, s * SJN + base: s * SJN + base + w],
                            in_=o_t[:, base: base + w]))
                        keep_pref[d_out.ins.name] = pref
                        all_dmas.append(d_out)
                elif s < NSTRIP - 1:
                    d_out = emit(P, nc.gpsimd.dma_start(
                        out=u[:, s * SJN: (s + 1) * SJN], in_=o_t[:, :]))
                    keep_pref[d_out.ins.name] = "Activation"
                    all_dmas.append(d_out)
                else:
                    # last strip: out-DMA per ACT copy (the first also
                    # carries the DVE region, covered by the pdum)
                    for j, (cp, cb, cw) in enumerate(acopies):
                        lo = 0 if j == 0 else cb
                        hi = cb + cw
                        d_out = emit(P, nc.gpsimd.dma_start(
                            out=u[:, s * SJN + lo: s * SJN + hi],
                            in_=o_t[:, lo: hi]))
                        keep_pref[d_out.ins.name] = "Activation"
                        all_dmas.append(d_out)

            # tail parking: cover DMAs + engine tails so the kernel-tail
            # drain has at most one wait left.  SWDGE (Pool) DMA sems only
            # have race-free wait values at their FINAL cumulative count, so
            # park just the last Pool DMA per DMASW lane.
            pool_dmas = [d for d in all_dmas
                         if d.ins.engine == ENG[P].engine]
            park_pool = set()
            for lane in range(8):
                lst = pool_dmas[lane::8]
                if lst:
                    park_pool.add(id(lst[-1]))
            parked = [d for d in all_dmas
                      if d.ins.engine != ENG[P].engine
                      or id(d) in park_pool]
            prev = None
            tails = [op for op in (last_eng.get(k) for k in
                                   ("tensor", V, A)) if op is not None]
            for d in parked + tails:
                w = nc.sync.nop(nofuse=True, hint="park")
                tile.add_dep_helper(w.ins, d.ins, sync=True, reason="park")
                if prev is not None:
                    tile.add_dep_helper(w.ins, prev.ins, sync=False,
                                        reason="ord")
                prev = w

    if not legalize:
        return nc

    # Single-wait legalization: for each multi-wait DMA keep the designated
    # wait — the dropped waits are implied by it through the dummy-op
    # ordering chains (the kept tick is only reached after the dropped
    # dependencies completed).
    import concourse.mybir as mybir2

    for blk in nc.m.functions[0].blocks:
        for inst in blk.instructions:
            si = inst.sync_info
            if si is None or not si.on_wait or len(si.on_wait) < 2:
                continue
            if type(inst).__name__ != "InstDMACopy":
                dbg = inst.debug
                raise RuntimeError(
                    f"unexpected multi-wait {inst.name} "
                    f"({type(inst).__name__} engine={inst.engine} "
                    f"line={getattr(dbg, 'lineno', None)}) "
                    f"{[w.ant_name for w in si.on_wait]}"
                )
            pref = keep_pref.get(inst.name)
            if pref is None:
                raise RuntimeError(
                    f"multi-wait DMA {inst.name} with no keep rule: "
                    f"{[w.ant_name for w in si.on_wait]}"
                )
            keep = [w for w in si.on_wait if w.ant_name.startswith(pref)]
            if len(keep) != 1:
                raise RuntimeError(
                    f"{inst.name}: expected one {pref} wait, got "
                    f"{[w.ant_name for w in si.on_wait]}"
                )
            inst.sync_info = mybir2.SyncInfo(
                on_wait=keep, on_update=list(si.on_update or [])
            )
    return nc


def _get_program():
    if "nc" not in _CACHE:
        _CACHE["nc"] = _build_program()
    return _CACHE["nc"]


def _bf16(a):
    import ml_dtypes
    return np.asarray(a, dtype=np.float32).astype(ml_dtypes.bfloat16)


def _host_prep(input, W):
    """Build per-core in_maps. input: [B, I, M]; W: [1, J, I, N, M]."""
    x = np.ascontiguousarray(input, dtype=np.float32)
    W0 = np.ascontiguousarray(W[0], dtype=np.float32)  # [J, I, N, M]

    # mask[(il, m), (il', bl)] = 1 iff il == il'
    il_row = (np.arange(128) // M)[:, None]
    il_col = (np.arange(128) // BL)[None, :]
    mask = _bf16((il_row == il_col).astype(np.float32))

    wts = []
    for jg in range(2):
        ws = W0[JL * jg: JL * jg + JL]                  # [JL, I, N, M]
        wts.append(_bf16(ws.transpose(1, 3, 0, 2).reshape(NCHUNK, 128, JN)))
    xcs = []
    for bg in range(4):
        xs = x[BL * bg: BL * bg + BL]                   # [BL, I, M]
        # xc[g, (il, m), bl] = x[bl, 16g+il, m]
        xcs.append(_bf16(xs.transpose(1, 2, 0).reshape(NCHUNK, 128, BL)))

    in_maps = []
    for c in range(NCORES):
        jg, bg = c % 2, c // 2
        in_maps.append(
            {"wb": wts[jg],
             "xb": np.ascontiguousarray(xcs[bg].transpose(1, 0, 2)),
             "msk": mask}
        )
    return in_maps


def _host_finish(input, results):
    """Gather selected child capsules and unshard over (j, b)."""
    mask = input.sum(axis=2) != 0.0                     # [B, I]
    keyv = np.where(mask, np.arange(I)[None, :], I)
    sidx = np.sort(keyv, axis=1)[:, :NZC]               # [B, NZC]

    ufull = np.empty((B, I, J, N), dtype=np.float32)
    for c in range(NCORES):
        jg, bg = c % 2, c // 2
        uc = np.asarray(results[c]["u"], dtype=np.float32)
        # u columns are chunk-major: [strip, chunk-in-strip, jl, n]
        uc = uc.reshape(16, BL, NCHUNK, JL, N)
        # partition p = 8*il + bl; i = 16*chunk + il
        uc = uc.transpose(1, 2, 0, 3, 4).reshape(BL, I, JL, N)
        ufull[BL * bg: BL * bg + BL, :, JL * jg: JL * jg + JL, :] = uc
    sel = ufull[np.arange(B)[:, None], sidx]            # [B, NZC, J, N]
    return np.ascontiguousarray(sel.transpose(0, 2, 1, 3))  # [B, J, NZC, N]


def run_on_cores(input, W, trace=False, **trace_kwargs):
    from concourse.bass_utils import run_bass_kernel_spmd

    nc = _get_program()
    in_maps = _host_prep(input, W)
    res = run_bass_kernel_spmd(
        nc, in_maps, list(range(NCORES)), trace=trace, **trace_kwargs
    )
    return _host_finish(input, res.results), res


def kernel(input, W):
    out, _ = run_on_cores(input, W)
    return out


if __name__ == "__main__":
    nc = _get_program()
    n_inst = sum(len(b.instructions) for b in nc.m.functions[0].blocks)
    print(f"built OK: {n_inst} instructions")
    from concourse.bass_interp import CoreSim
    import reference as R
    import jax
    with jax.default_device(jax.devices("cpu")[0]):
        inputs = {k: np.asarray(v) for k, v in R.setup_inputs().items()}
    sim = CoreSim(nc)
    sim.assign_tensors(_host_prep(inputs["input"], inputs["W"])[0])
    sim.simulate()
    print(f"sim time: {sim.time} ns")


# revision 45
# speedup vs baseline: 2.7830x; 1.0124x over previous
"""Trainium2 Bass kernel for nn_DenseCapsuleLayer.

Reference computation:
    u_hat[b, j, k, n] = sum_m W[0, j, idx[b,k], n, m] * x[b, idx[b,k], m]
with idx[b, :] the ascending indices of the NZC=1152 non-zero child capsules
of batch b (x is zero elsewhere).

Strategy (8 NeuronCores, 2-way parent-capsule x 4-way batch mesh):
  * Core c owns j in [16*(c%2), 16*(c%2)+16) and b in [8*(c//2), 8*(c//2)+8).
  * Each core computes the DENSE map u_full[b, i, jl, n] for ALL i in bf16
    (x is zero at non-selected i so u_full there is zero and discarded);
    the select/compaction gather over i and the unshard happen on the host.
  * Per 16-wide child-capsule chunk g (i = 16g+il), the PE computes
        out[(il,bl), (jl,n)] = sum_m x[b, 16g+il, m] * W[j, 16g+il, n, m]
    as ONE K=128 bf16 matmul: stationary = [128,128] block-diagonal packing
    of the core's x slice (8 batches), built on device by ONE batched
    broadcast-multiply per strip against a static 0/1 mask (built one strip
    AHEAD so the PE is never gated on the builder); moving = the core's W
    slice pre-transposed to [(il,m), (jl,n)] (256 free columns).
  * Everything over DMA is bf16 (W, x, mask, output) halving HBM bytes;
    PSUM accumulates f32; the PSUM->SBUF copies cast f32->bf16.
  * The CoreSim cost model charges each DMA's transfer to the ISSUING
    engine's queue and queues run concurrently, so the work is spread:
    SP issues most strip in-DMAs (the first three are split across
    SP/ACT/Pool so the pipeline fills fast); the 36 PSUM->SBUF cast
    copies are split DVE/ACT (1/2 per strip — GPSIMD cannot touch PSUM);
    out-DMAs leave per copy-engine region, mostly on the Pool queue.
  * The output staging tiles (o_v/o_a) and the stationaries (bdt) are NOT
    pool-recycled: each strip gets its own SBUF tile, which deletes every
    slot-reuse WAR/WAW hazard (and its dummy-op tax) on the copy engines.
    Only the W stream (3 slots) and PSUM (4 quad slots) recycle.

Toolchain constraints: every lowered instruction accepts ONE sync-wait
command, Tile emits a wait per dependency it cannot prove covered, and its
coverage tracking credits only REAL data dependencies.  Per strip, dummy
ops each carry one wait: sdumv (DVE) reads the fresh W strip so the
stationary-builder needs no DMA wait, and on PE zero-cost bare ldweights
reads absorb the in-DMA (sdum), the stationary RAW (bdtdum), and the
PSUM-slot WAR (gdum reads the o region the bank-freeing copy wrote).
Matmuls may carry a PE self-wait (PSUM slot WAW), copies carry their PE
RAW, out-DMAs carry their copy engine's tick.  SP nops park the
kernel-tail drain's wait list, and a BIR post-pass drops the in-DMA waits
(W-slot WAR) that are provably implied by the kept PE wait.
"""

import numpy as np

B, I, J, M, N = 32, 2304, 32, 8, 16
NZC = I // 2
NCORES = 8
JL = J // 2               # parent capsules per core (16)
JN = JL * N               # 256
BL = B // 4               # batches per core (8)
NCHUNK = I // 16          # 144 chunks of 16 child capsules
NSTRIP = 18
G = NCHUNK // NSTRIP      # 8 chunks per strip
# DVE/ACT copy split alternates (2,6)/(4,4) by strip parity: PSUM tags
# psA0[2]+psA1[4]+psB0[6]+psB1[4] = exactly 8 banks, and the average
# balances DVE (which also builds stationaries) against ACT.
GVP = [4, 2]              # chunks drained by DVE, by strip parity
SJN = G * JN              # 2048 columns per strip in u

V, A, P = "vector", "scalar", "gpsimd"

_CACHE = {}


def _build_program(legalize=True):
    import concourse.bass as bass
    import concourse.mybir as mybir
    import concourse.tile as tile

    f32 = mybir.dt.float32
    bf16 = mybir.dt.bfloat16
    nc = bass.Bass()

    # wb[g, (il,m), 0:256] = W[j, 16g+il, n, m]  (moving operand)
    wb = nc.declare_dram_parameter("wb", [NCHUNK, 128, JN], bf16,
                                   isOutput=False)
    # xb[(il,m), g, bl] = x[bl, 16g+il, m] for the core's 8 batches
    xb = nc.declare_dram_parameter("xb", [128, NCHUNK, BL], bf16,
                                   isOutput=False)
    # msk[(il,m), (il',bl)] = 1.0 iff il == il'
    msk = nc.declare_dram_parameter("msk", [128, 128], bf16, isOutput=False)
    u = nc.declare_dram_parameter("u", [128, NSTRIP * SJN], bf16,
                                  isOutput=True)

    keep_pref = {}   # DMA inst name -> sem prefix of the wait to keep

    with tile.TileContext(nc, pool_alloc_mode="queue") as tc:
        with (
            tc.tile_pool(name="wpool", bufs=1) as wpool,
            tc.tile_pool(name="ppool", bufs=1, space="PSUM") as ppool,
            tc.tile_pool(name="zpool", bufs=1) as zpool,
        ):
            ENG = {V: nc.vector, A: nc.scalar, P: nc.gpsimd, "sync": nc.sync}
            last_on = {}          # engine key -> last op (forced order)
            last_eng = {}         # engine key -> last non-DMA engine op

            def emit(key, op):
                prev = last_on.get(key)
                if prev is not None:
                    tile.add_dep_helper(op.ins, prev.ins, sync=False,
                                        reason="ord")
                last_on[key] = op
                if type(op.ins).__name__ != "InstDMACopy":
                    last_eng[key] = op
                return op

            def ecopy(key, dst, src):
                if key == A:
                    return emit(A, nc.scalar.copy(dst, src))
                return emit(key, ENG[key].tensor_copy(dst, src))

            mask_t = zpool.tile([128, 128], bf16, tag="msk")
            scr_v = zpool.tile([1, 32], bf16, tag="scr0")
            scr_p = zpool.tile([1, 32], bf16, tag="scr1")
            scr_a = zpool.tile([1, 8], bf16, tag="scr2")
            pcol = [0]

            def pcell():
                c = pcol[0]
                pcol[0] += 1
                assert c < 32
                return scr_p[0:1, c: c + 1]
            scol = [0]

            def vcell():
                c = scol[0]
                scol[0] += 1
                assert c < 32
                return scr_v[0:1, c: c + 1]

            x_sb = zpool.tile([128, NCHUNK, BL], bf16, tag="x")
            # per-strip unpooled tiles (no slot reuse -> no WAR/WAW tax);
            # one o tile per strip: DVE writes cols [0:VJN), ACT the rest,
            # and a single out-DMA moves the whole strip
            o_ts, bdts = [], []
            for s in range(NSTRIP):
                t1 = zpool.tile([128, SJN], bf16, tag=f"o{s}")
                t3 = zpool.tile([128, G, 128], bf16, tag=f"bd{s}")
                o_ts.append(t1)
                bdts.append(t3)

            all_dmas = []

            # --- startup ---------------------------------------------------
            d_msk = emit(A, nc.scalar.dma_start(out=mask_t[:, :],
                                                in_=msk[:, :]))
            all_dmas.append(d_msk)
            # x ships once, early, on the ACT queue
            d_x = emit(A, nc.scalar.dma_start(out=x_sb[:, :, :],
                                              in_=xb[:, :, :]))
            all_dmas.append(d_x)
            # absorb the mask and x ticks on DVE (the only consumers)
            emit(V, nc.vector.tensor_copy(vcell(), mask_t[0:1, 0:1]))
            emit(V, nc.vector.tensor_copy(vcell(), x_sb[0:1, 0, 0:1]))

            mask4 = mask_t.rearrange("p (s r c) -> p s r c", s=1, r=16)

            w_tiles = []

            def prefetch(s, qk="sync", halves=False):
                w_sb = wpool.tile([128, G, JN], bf16, tag=f"w{s % 5}")
                w_tiles.append(w_sb)
                if halves:
                    h = G // 2
                    for qq, lo in (("sync", 0), (A, h)):
                        d_in = emit(qq, ENG[qq].dma_start(
                            out=w_sb[:, lo: lo + h, :],
                            in_=wb[s * G + lo: s * G + lo + h].rearrange(
                                "g p c -> p g c"),
                        ))
                        keep_pref[d_in.ins.name] = "PE"
                        all_dmas.append(d_in)
                else:
                    d_in = emit(qk, ENG[qk].dma_start(
                        out=w_sb[:, :, :],
                        in_=wb[s * G: (s + 1) * G].rearrange("g p c -> p g c"),
                    ))
                    keep_pref[d_in.ins.name] = "PE"
                    all_dmas.append(d_in)

            def build_bdt(s):
                """Stationary build for strip s (reads x_sb + mask)."""
                x4 = x_sb[:, s * G: (s + 1) * G, :].rearrange(
                    "p g (s c) -> p g s c", s=1).broadcast_to([128, G, 16, BL])
                emit(V, nc.vector.tensor_mul(
                    bdts[s].rearrange("p g (r c) -> p g r c", r=16),
                    x4,
                    mask4.broadcast_to([128, G, 16, BL]),
                ))

            # split the first strip's load across two queues so the pipeline
            # fills fast; spread the next two over otherwise-idle queues
            prefetch(0, halves=True)
            prefetch(1, "sync")
            prefetch(2, "sync")
            prefetch(3, "sync")
            # ACT warmup: pays the activation-table load during startup
            # idle, after the startup DMAs ACT issues
            emit(A, nc.scalar.copy(scr_a[0:1, 0:1], x_sb[0:1, 0, 0:1]))
            build_bdt(0)

            for s in range(NSTRIP):
                w_sb = w_tiles[s]
                o_t, bdt = o_ts[s], bdts[s]

                # absorb the strip in-DMA tick on PE, then the stationary
                # builder's tick (both zero-cost bare weight loads)
                emit("tensor", nc.tensor.ldweights(w_sb[0:32, 0, 0:1]))
                emit("tensor", nc.tensor.ldweights(bdt[0:32, 0, 0:1]))

                # build the NEXT strip's stationaries before this strip's
                # DVE copy so the PE is never gated on the builder
                if s + 1 < NSTRIP:
                    build_bdt(s + 1)

                acopies = []
                gv = GVP[s % 2]
                vjn = gv * JN
                for ek, base, gn in ((A, vjn, G - gv), (V, 0, gv)):
                    tagc = "A" if ek == V else "B"
                    ps = ppool.tile([128, gn, JN], f32,
                                    tag=f"ps{tagc}{s % 2}")
                    assert gn == (GVP[s % 2] if ek == V else G - GVP[s % 2])
                    if s >= 2:
                        # gdum: read the o region the bank-freeing copy (same
                        # slot, two strips back) wrote — the cross-engine
                        # wait covers the PSUM-slot WAR
                        emit("tensor", nc.tensor.ldweights(
                            o_ts[s - 2][0:32, base: base + 1]))
                    for h in range(gn):
                        gl = (0 if ek == V else gv) + h
                        emit("tensor", nc.tensor.matmul(
                            ps[:, h, :], bdt[:, gl, :], w_sb[:, gl, 0:JN],
                            start=True, stop=True,
                        ))
                    if ek == V or s < NSTRIP - 1:
                        cp = ecopy(ek, o_t[:, base: base + gn * JN],
                                   ps.rearrange("p a b -> p (a b)"))
                        if ek == A:
                            acopies.append((cp, base, gn * JN))
                    else:
                        # last strip: small 2-chunk ACT copies so the final
                        # copy->out chain is short
                        for j in range(gn // 2):
                            cp = ecopy(A, o_t[:, base + 2 * j * JN:
                                              base + 2 * (j + 1) * JN],
                                       ps[:, 2 * j: 2 * j + 2, :].rearrange(
                                           "p a b -> p (a b)"))
                            acopies.append((cp, base + 2 * j * JN, 2 * JN))

                if s + 4 < NSTRIP:
                    prefetch(s + 4)

                # pdum: near-free Pool op reading the DVE region — its DVE
                # wait lets the strip out-DMA(s) carry only the ACT wait
                emit(P, nc.gpsimd.tensor_copy(pcell(), o_t[0:1, 0:1]))
                if s in (NSTRIP - 3, NSTRIP - 2):
                    # late strips go out on SP (idle by then) as separate
                    # V/A-region DMAs so each carries one engine wait
                    for base, w, pref in ((0, vjn, "DVE"),
                                          (vjn, SJN - vjn, "Activation")):
                        d_out = emit("sync", nc.sync.dma_start(
                            out=u[:, s * SJN + base: s * SJN + base + w],
                            in_=o_t[:, base: base + w]))
                        keep_pref[d_out.ins.name] = pref
                        all_dmas.append(d_out)
                elif s < NSTRIP - 1:
                    d_out = emit(P, nc.gpsimd.dma_start(
                        out=u[:, s * SJN: (s + 1) * SJN], in_=o_t[:, :]))
                    keep_pref[d_out.ins.name] = "Activation"
                    all_dmas.append(d_out)
                else:
                    # last strip: out-DMA per ACT copy (the first also
                    # carries the DVE region, covered by the pdum); the
                    # later ones issue from ACT itself, which is idle and
                    # needs no cross-engine wait for its own copies
                    for j, (cp, cb, cw) in enumerate(acopies):
                        lo = 0 if j == 0 else cb
                        hi = cb + cw
                        qk = P if j == 0 else A
                        d_out = emit(qk, ENG[qk].dma_start(
                            out=u[:, s * SJN + lo: s * SJN + hi],
                            in_=o_t[:, lo: hi]))
                        keep_pref[d_out.ins.name] = "Activation"
                        all_dmas.append(d_out)

            # tail parking: cover DMAs + engine tails so the kernel-tail
            # drain has at most one wait left.  SWDGE (Pool) DMA sems only
            # have race-free wait values at their FINAL cumulative count, so
            # park just the last Pool DMA per DMASW lane.
            pool_dmas = [d for d in all_dmas
                         if d.ins.engine == ENG[P].engine]
            park_pool = set()
            for lane in range(8):
                lst = pool_dmas[lane::8]
                if lst:
                    park_pool.add(id(lst[-1]))
            parked = [d for d in all_dmas
                      if d.ins.engine != ENG[P].engine
                      or id(d) in park_pool]
            prev = None
            tails = [op for op in (last_eng.get(k) for k in
                                   ("tensor", V, A)) if op is not None]
            for d in parked + tails:
                w = nc.sync.nop(nofuse=True, hint="park")
                tile.add_dep_helper(w.ins, d.ins, sync=True, reason="park")
                if prev is not None:
                    tile.add_dep_helper(w.ins, prev.ins, sync=False,
                                        reason="ord")
                prev = w

    if not legalize:
        return nc

    # Single-wait legalization: for each multi-wait DMA keep the designated
    # wait — the dropped waits are implied by it through the dummy-op
    # ordering chains (the kept tick is only reached after the dropped
    # dependencies completed).
    import concourse.mybir as mybir2

    for blk in nc.m.functions[0].blocks:
        for inst in blk.instructions:
            si = inst.sync_info
            if si is None or not si.on_wait or len(si.on_wait) < 2:
                continue
            if type(inst).__name__ != "InstDMACopy":
                dbg = inst.debug
                raise RuntimeError(
                    f"unexpected multi-wait {inst.name} "
                    f"({type(inst).__name__} engine={inst.engine} "
                    f"line={getattr(dbg, 'lineno', None)}) "
                    f"{[w.ant_name for w in si.on_wait]}"
                )
            pref = keep_pref.get(inst.name)
            if pref is None:
                raise RuntimeError(
                    f"multi-wait DMA {inst.name} with no keep rule: "
                    f"{[w.ant_name for w in si.on_wait]}"
                )
            keep = [w for w in si.on_wait if w.ant_name.startswith(pref)]
            if len(keep) != 1:
                raise RuntimeError(
                    f"{inst.name}: expected one {pref} wait, got "
                    f"{[w.ant_name for w in si.on_wait]}"
                )
            inst.sync_info = mybir2.SyncInfo(
                on_wait=keep, on_update=list(si.on_update or [])
            )
    return nc


def _get_program():
    if "nc" not in _CACHE:
        _CACHE["nc"] = _build_program()
    return _CACHE["nc"]


def _bf16(a):
    import ml_dtypes
    return np.asarray(a, dtype=np.float32).astype(ml_dtypes.bfloat16)


def _host_prep(input, W):
    """Build per-core in_maps. input: [B, I, M]; W: [1, J, I, N, M]."""
    x = np.ascontiguousarray(input, dtype=np.float32)
    W0 = np.ascontiguousarray(W[0], dtype=np.float32)  # [J, I, N, M]

    # mask[(il, m), (il', bl)] = 1 iff il == il'
    il_row = (np.arange(128) // M)[:, None]
    il_col = (np.arange(128) // BL)[None, :]
    mask = _bf16((il_row == il_col).astype(np.float32))

    wts = []
    for jg in range(2):
        ws = W0[JL * jg: JL * jg + JL]                  # [JL, I, N, M]
        wts.append(_bf16(ws.transpose(1, 3, 0, 2).reshape(NCHUNK, 128, JN)))
    xcs = []
    for bg in range(4):
        xs = x[BL * bg: BL * bg + BL]                   # [BL, I, M]
        # xc[g, (il, m), bl] = x[bl, 16g+il, m]
        xcs.append(_bf16(xs.transpose(1, 2, 0).reshape(NCHUNK, 128, BL)))

    in_maps = []
    for c in range(NCORES):
        jg, bg = c % 2, c // 2
        in_maps.append(
            {"wb": wts[jg],
             "xb": np.ascontiguousarray(xcs[bg].transpose(1, 0, 2)),
             "msk": mask}
        )
    return in_maps


def _host_finish(input, results):
    """Gather selected child capsules and unshard over (j, b)."""
    mask = input.sum(axis=2) != 0.0                     # [B, I]
    keyv = np.where(mask, np.arange(I)[None, :], I)
    sidx = np.sort(keyv, axis=1)[:, :NZC]               # [B, NZC]

    ufull = np.empty((B, I, J, N), dtype=np.float32)
    for c in range(NCORES):
        jg, bg = c % 2, c // 2
        uc = np.asarray(results[c]["u"], dtype=np.float32)
        # u columns are chunk-major: [strip, chunk-in-strip, jl, n]
        uc = uc.reshape(16, BL, NCHUNK, JL, N)
        # partition p = 8*il + bl; i = 16*chunk + il
        uc = uc.transpose(1, 2, 0, 3, 4).reshape(BL, I, JL, N)
        ufull[BL * bg: BL * bg + BL, :, JL * jg: JL * jg + JL, :] = uc
    sel = ufull[np.arange(B)[:, None], sidx]            # [B, NZC, J, N]
    return np.ascontiguousarray(sel.transpose(0, 2, 1, 3))  # [B, J, NZC, N]


def run_on_cores(input, W, trace=False, **trace_kwargs):
    from concourse.bass_utils import run_bass_kernel_spmd

    nc = _get_program()
    in_maps = _host_prep(input, W)
    res = run_bass_kernel_spmd(
        nc, in_maps, list(range(NCORES)), trace=trace, **trace_kwargs
    )
    return _host_finish(input, res.results), res


def kernel(input, W):
    out, _ = run_on_cores(input, W)
    return out


if __name__ == "__main__":
    nc = _get_program()
    n_inst = sum(len(b.instructions) for b in nc.m.functions[0].blocks)
    print(f"built OK: {n_inst} instructions")
    from concourse.bass_interp import CoreSim
    import reference as R
    import jax
    with jax.default_device(jax.devices("cpu")[0]):
        inputs = {k: np.asarray(v) for k, v in R.setup_inputs().items()}
    sim = CoreSim(nc)
    sim.assign_tensors(_host_prep(inputs["input"], inputs["W"])[0])
    sim.simulate()
    print(f"sim time: {sim.time} ns")


# revision 50
# speedup vs baseline: 2.8148x; 1.0114x over previous
"""Trainium2 Bass kernel for nn_DenseCapsuleLayer.

Reference computation:
    u_hat[b, j, k, n] = sum_m W[0, j, idx[b,k], n, m] * x[b, idx[b,k], m]
with idx[b, :] the ascending indices of the NZC=1152 non-zero child capsules
of batch b (x is zero elsewhere).

Strategy (8 NeuronCores, 2-way parent-capsule x 4-way batch mesh):
  * Core c owns j in [16*(c%2), 16*(c%2)+16) and b in [8*(c//2), 8*(c//2)+8).
  * Each core computes the DENSE map u_full[b, i, jl, n] for ALL i in bf16
    (x is zero at non-selected i so u_full there is zero and discarded);
    the select/compaction gather over i and the unshard happen on the host.
  * Per 16-wide child-capsule chunk g (i = 16g+il), the PE computes
        out[(il,bl), (jl,n)] = sum_m x[b, 16g+il, m] * W[j, 16g+il, n, m]
    as ONE K=128 bf16 matmul: stationary = [128,128] block-diagonal packing
    of the core's x slice (8 batches), built on device by ONE batched
    broadcast-multiply per strip against a static 0/1 mask (built one strip
    AHEAD so the PE is never gated on the builder); moving = the core's W
    slice pre-transposed to [(il,m), (jl,n)] (256 free columns).
  * Everything over DMA is bf16 (W, x, mask, output) halving HBM bytes;
    PSUM accumulates f32; the PSUM->SBUF copies cast f32->bf16.
  * The CoreSim cost model charges each DMA's transfer to the ISSUING
    engine's queue and queues run concurrently, so the work is spread:
    SP issues most strip in-DMAs (the first three are split across
    SP/ACT/Pool so the pipeline fills fast); the 36 PSUM->SBUF cast
    copies are split DVE/ACT (1/2 per strip — GPSIMD cannot touch PSUM);
    out-DMAs leave per copy-engine region, mostly on the Pool queue.
  * The output staging tiles (o_v/o_a) and the stationaries (bdt) are NOT
    pool-recycled: each strip gets its own SBUF tile, which deletes every
    slot-reuse WAR/WAW hazard (and its dummy-op tax) on the copy engines.
    Only the W stream (3 slots) and PSUM (4 quad slots) recycle.

Toolchain constraints: every lowered instruction accepts ONE sync-wait
command, Tile emits a wait per dependency it cannot prove covered, and its
coverage tracking credits only REAL data dependencies.  Per strip, dummy
ops each carry one wait: sdumv (DVE) reads the fresh W strip so the
stationary-builder needs no DMA wait, and on PE zero-cost bare ldweights
reads absorb the in-DMA (sdum), the stationary RAW (bdtdum), and the
PSUM-slot WAR (gdum reads the o region the bank-freeing copy wrote).
Matmuls may carry a PE self-wait (PSUM slot WAW), copies carry their PE
RAW, out-DMAs carry their copy engine's tick.  SP nops park the
kernel-tail drain's wait list, and a BIR post-pass drops the in-DMA waits
(W-slot WAR) that are provably implied by the kept PE wait.
"""

import numpy as np

B, I, J, M, N = 32, 2304, 32, 8, 16
NZC = I // 2
NCORES = 8
JL = J // 2               # parent capsules per core (16)
JN = JL * N               # 256
BL = B // 4               # batches per core (8)
NCHUNK = I // 16          # 144 chunks of 16 child capsules
NSTRIP = 18
G = NCHUNK // NSTRIP      # 8 chunks per strip
# DVE/ACT copy split alternates (2,6)/(4,4) by strip parity: PSUM tags
# psA0[2]+psA1[4]+psB0[6]+psB1[4] = exactly 8 banks, and the average
# balances DVE (which also builds stationaries) against ACT.
GVP = [4, 2]              # chunks drained by DVE, by strip parity
SJN = G * JN              # 2048 columns per strip in u

V, A, P = "vector", "scalar", "gpsimd"

_CACHE = {}


def _build_program(legalize=True):
    import concourse.bass as bass
    import concourse.mybir as mybir
    import concourse.tile as tile

    f32 = mybir.dt.float32
    bf16 = mybir.dt.bfloat16
    nc = bass.Bass()

    # wb[g, (il,m), 0:256] = W[j, 16g+il, n, m]  (moving operand)
    wb = nc.declare_dram_parameter("wb", [NCHUNK, 128, JN], bf16,
                                   isOutput=False)
    # xb[(il,m), g, bl] = x[bl, 16g+il, m] for the core's 8 batches
    xb = nc.declare_dram_parameter("xb", [128, NCHUNK, BL], bf16,
                                   isOutput=False)
    # msk[(il,m), (il',bl)] = 1.0 iff il == il'
    msk = nc.declare_dram_parameter("msk", [128, 128], bf16, isOutput=False)
    u = nc.declare_dram_parameter("u", [128, NSTRIP * SJN], bf16,
                                  isOutput=True)

    keep_pref = {}   # DMA inst name -> sem prefix of the wait to keep

    with tile.TileContext(nc, pool_alloc_mode="queue") as tc:
        with (
            tc.tile_pool(name="wpool", bufs=1) as wpool,
            tc.tile_pool(name="ppool", bufs=1, space="PSUM") as ppool,
            tc.tile_pool(name="zpool", bufs=1) as zpool,
        ):
            ENG = {V: nc.vector, A: nc.scalar, P: nc.gpsimd, "sync": nc.sync}
            last_on = {}          # engine key -> last op (forced order)
            last_eng = {}         # engine key -> last non-DMA engine op

            def emit(key, op):
                prev = last_on.get(key)
                if prev is not None:
                    tile.add_dep_helper(op.ins, prev.ins, sync=False,
                                        reason="ord")
                last_on[key] = op
                if type(op.ins).__name__ != "InstDMACopy":
                    last_eng[key] = op
                return op

            def ecopy(key, dst, src):
                if key == A:
                    return emit(A, nc.scalar.copy(dst, src))
                return emit(key, ENG[key].tensor_copy(dst, src))

            mask_t = zpool.tile([128, 128], bf16, tag="msk")
            scr_v = zpool.tile([1, 32], bf16, tag="scr0")
            scr_p = zpool.tile([1, 32], bf16, tag="scr1")
            scr_a = zpool.tile([1, 8], bf16, tag="scr2")
            pcol = [0]

            def pcell():
                c = pcol[0]
                pcol[0] += 1
                assert c < 32
                return scr_p[0:1, c: c + 1]
            scol = [0]

            def vcell():
                c = scol[0]
                scol[0] += 1
                assert c < 32
                return scr_v[0:1, c: c + 1]

            x_sb = zpool.tile([128, NCHUNK, BL], bf16, tag="x")
            # per-strip unpooled tiles (no slot reuse -> no WAR/WAW tax);
            # one o tile per strip: DVE writes cols [0:VJN), ACT the rest,
            # and a single out-DMA moves the whole strip
            o_ts, bdts = [], []
            for s in range(NSTRIP):
                t1 = zpool.tile([128, SJN], bf16, tag=f"o{s}")
                t3 = zpool.tile([128, G, 128], bf16, tag=f"bd{s}")
                o_ts.append(t1)
                bdts.append(t3)

            all_dmas = []

            # --- startup ---------------------------------------------------
            d_msk = emit(P, nc.gpsimd.dma_start(out=mask_t[:, :],
                                                in_=msk[:, :]))
            all_dmas.append(d_msk)
            # x ships once, early, on the Pool queue
            d_x = emit(P, nc.gpsimd.dma_start(out=x_sb[:, :, :],
                                              in_=xb[:, :, :]))
            all_dmas.append(d_x)
            # absorb the mask and x ticks on DVE (the only consumers)
            emit(V, nc.vector.tensor_copy(vcell(), mask_t[0:1, 0:1]))
            emit(V, nc.vector.tensor_copy(vcell(), x_sb[0:1, 0, 0:1]))

            mask4 = mask_t.rearrange("p (s r c) -> p s r c", s=1, r=16)

            w_tiles = []

            def prefetch(s, qk="sync", halves=False):
                w_sb = wpool.tile([128, G, JN], bf16, tag=f"w{s % 6}")
                w_tiles.append(w_sb)
                if halves:
                    h = G // 2
                    for qq, lo in (("sync", 0), (A, h)):
                        d_in = emit(qq, ENG[qq].dma_start(
                            out=w_sb[:, lo: lo + h, :],
                            in_=wb[s * G + lo: s * G + lo + h].rearrange(
                                "g p c -> p g c"),
                        ))
                        keep_pref[d_in.ins.name] = "PE"
                        all_dmas.append(d_in)
                else:
                    d_in = emit(qk, ENG[qk].dma_start(
                        out=w_sb[:, :, :],
                        in_=wb[s * G: (s + 1) * G].rearrange("g p c -> p g c"),
                    ))
                    keep_pref[d_in.ins.name] = "PE"
                    all_dmas.append(d_in)

            def build_bdt(s):
                """Stationary build for strip s (reads x_sb + mask)."""
                x4 = x_sb[:, s * G: (s + 1) * G, :].rearrange(
                    "p g (s c) -> p g s c", s=1).broadcast_to([128, G, 16, BL])
                emit(V, nc.vector.tensor_mul(
                    bdts[s].rearrange("p g (r c) -> p g r c", r=16),
                    x4,
                    mask4.broadcast_to([128, G, 16, BL]),
                ))

            # split the first strip's load across two queues so the pipeline
            # fills fast; spread the next two over otherwise-idle queues
            prefetch(0, halves=True)
            prefetch(1, "sync")
            prefetch(2, "sync")
            prefetch(3, "sync")
            # ACT warmup: pays the activation-table load during startup
            # idle, after the startup DMAs ACT issues
            emit(A, nc.scalar.copy(scr_a[0:1, 0:1], x_sb[0:1, 0, 0:1]))
            build_bdt(0)

            for s in range(NSTRIP):
                w_sb = w_tiles[s]
                o_t, bdt = o_ts[s], bdts[s]

                # absorb the strip in-DMA tick on PE, then the stationary
                # builder's tick (both zero-cost bare weight loads)
                emit("tensor", nc.tensor.ldweights(w_sb[0:32, 0, 0:1]))
                emit("tensor", nc.tensor.ldweights(bdt[0:32, 0, 0:1]))

                # build the NEXT strip's stationaries before this strip's
                # DVE copy so the PE is never gated on the builder
                if s + 1 < NSTRIP:
                    build_bdt(s + 1)

                acopies = []
                gv = GVP[s % 2]
                vjn = gv * JN
                for ek, base, gn in ((A, vjn, G - gv), (V, 0, gv)):
                    tagc = "A" if ek == V else "B"
                    ps = ppool.tile([128, gn, JN], f32,
                                    tag=f"ps{tagc}{s % 2}")
                    assert gn == (GVP[s % 2] if ek == V else G - GVP[s % 2])
                    if s >= 2:
                        # gdum: read the o region the bank-freeing copy (same
                        # slot, two strips back) wrote — the cross-engine
                        # wait covers the PSUM-slot WAR
                        emit("tensor", nc.tensor.ldweights(
                            o_ts[s - 2][0:32, base: base + 1]))
                    for h in range(gn):
                        gl = (0 if ek == V else gv) + h
                        emit("tensor", nc.tensor.matmul(
                            ps[:, h, :], bdt[:, gl, :], w_sb[:, gl, 0:JN],
                            start=True, stop=True,
                        ))
                    if ek == V or s < NSTRIP - 1:
                        cp = ecopy(ek, o_t[:, base: base + gn * JN],
                                   ps.rearrange("p a b -> p (a b)"))
                        if ek == A:
                            acopies.append((cp, base, gn * JN))
                    else:
                        # last strip: small 2-chunk ACT copies so the final
                        # copy->out chain is short
                        for j in range(gn // 2):
                            cp = ecopy(A, o_t[:, base + 2 * j * JN:
                                              base + 2 * (j + 1) * JN],
                                       ps[:, 2 * j: 2 * j + 2, :].rearrange(
                                           "p a b -> p (a b)"))
                            acopies.append((cp, base + 2 * j * JN, 2 * JN))

                if s + 4 == NSTRIP - 14 + 4:
                    pass
                if s + 4 < NSTRIP:
                    prefetch(s + 4)

                # pdum: near-free Pool op reading the DVE region — its DVE
                # wait lets the strip out-DMA(s) carry only the ACT wait
                emit(P, nc.gpsimd.tensor_copy(pcell(), o_t[0:1, 0:1]))
                if s in (NSTRIP - 3, NSTRIP - 2):
                    # late strips go out on SP (idle by then) as separate
                    # V/A-region DMAs so each carries one engine wait
                    for base, w, pref in ((0, vjn, "DVE"),
                                          (vjn, SJN - vjn, "Activation")):
                        d_out = emit("sync", nc.sync.dma_start(
                            out=u[:, s * SJN + base: s * SJN + base + w],
                            in_=o_t[:, base: base + w]))
                        keep_pref[d_out.ins.name] = pref
                        all_dmas.append(d_out)
                elif s < NSTRIP - 1:
                    d_out = emit(P, nc.gpsimd.dma_start(
                        out=u[:, s * SJN: (s + 1) * SJN], in_=o_t[:, :]))
                    keep_pref[d_out.ins.name] = "Activation"
                    all_dmas.append(d_out)
                else:
                    # last strip: out-DMA per ACT copy (the first also
                    # carries the DVE region, covered by the pdum); the
                    # later ones issue from ACT itself, which is idle and
                    # needs no cross-engine wait for its own copies
                    for j, (cp, cb, cw) in enumerate(acopies):
                        lo = 0 if j == 0 else cb
                        hi = cb + cw
                        qk = P if j == 0 else A
                        d_out = emit(qk, ENG[qk].dma_start(
                            out=u[:, s * SJN + lo: s * SJN + hi],
                            in_=o_t[:, lo: hi]))
                        keep_pref[d_out.ins.name] = "Activation"
                        all_dmas.append(d_out)

            # tail parking: cover DMAs + engine tails so the kernel-tail
            # drain has at most one wait left.  SWDGE (Pool) DMA sems only
            # have race-free wait values at their FINAL cumulative count, so
            # park just the last Pool DMA per DMASW lane.
            pool_dmas = [d for d in all_dmas
                         if d.ins.engine == ENG[P].engine]
            park_pool = set()
            for lane in range(8):
                lst = pool_dmas[lane::8]
                if lst:
                    park_pool.add(id(lst[-1]))
            parked = [d for d in all_dmas
                      if d.ins.engine != ENG[P].engine
                      or id(d) in park_pool]
            prev = None
            tails = [op for op in (last_eng.get(k) for k in
                                   ("tensor", V, A)) if op is not None]
            for d in parked + tails:
                w = nc.sync.nop(nofuse=True, hint="park")
                tile.add_dep_helper(w.ins, d.ins, sync=True, reason="park")
                if prev is not None:
                    tile.add_dep_helper(w.ins, prev.ins, sync=False,
                                        reason="ord")
                prev = w

    if not legalize:
        return nc

    # Single-wait legalization: for each multi-wait DMA keep the designated
    # wait — the dropped waits are implied by it through the dummy-op
    # ordering chains (the kept tick is only reached after the dropped
    # dependencies completed).
    import concourse.mybir as mybir2

    for blk in nc.m.functions[0].blocks:
        for inst in blk.instructions:
            si = inst.sync_info
            if si is None or not si.on_wait or len(si.on_wait) < 2:
                continue
            if type(inst).__name__ != "InstDMACopy":
                dbg = inst.debug
                raise RuntimeError(
                    f"unexpected multi-wait {inst.name} "
                    f"({type(inst).__name__} engine={inst.engine} "
                    f"line={getattr(dbg, 'lineno', None)}) "
                    f"{[w.ant_name for w in si.on_wait]}"
                )
            pref = keep_pref.get(inst.name)
            if pref is None:
                raise RuntimeError(
                    f"multi-wait DMA {inst.name} with no keep rule: "
                    f"{[w.ant_name for w in si.on_wait]}"
                )
            keep = [w for w in si.on_wait if w.ant_name.startswith(pref)]
            if len(keep) != 1:
                raise RuntimeError(
                    f"{inst.name}: expected one {pref} wait, got "
                    f"{[w.ant_name for w in si.on_wait]}"
                )
            inst.sync_info = mybir2.SyncInfo(
                on_wait=keep, on_update=list(si.on_update or [])
            )
    return nc


def _get_program():
    if "nc" not in _CACHE:
        _CACHE["nc"] = _build_program()
    return _CACHE["nc"]


def _bf16(a):
    import ml_dtypes
    return np.asarray(a, dtype=np.float32).astype(ml_dtypes.bfloat16)


def _host_prep(input, W):
    """Build per-core in_maps. input: [B, I, M]; W: [1, J, I, N, M]."""
    x = np.ascontiguousarray(input, dtype=np.float32)
    W0 = np.ascontiguousarray(W[0], dtype=np.float32)  # [J, I, N, M]

    # mask[(il, m), (il', bl)] = 1 iff il == il'
    il_row = (np.arange(128) // M)[:, None]
    il_col = (np.arange(128) // BL)[None, :]
    mask = _bf16((il_row == il_col).astype(np.float32))

    wts = []
    for jg in range(2):
        ws = W0[JL * jg: JL * jg + JL]                  # [JL, I, N, M]
        wts.append(_bf16(ws.transpose(1, 3, 0, 2).reshape(NCHUNK, 128, JN)))
    xcs = []
    for bg in range(4):
        xs = x[BL * bg: BL * bg + BL]                   # [BL, I, M]
        # xc[g, (il, m), bl] = x[bl, 16g+il, m]
        xcs.append(_bf16(xs.transpose(1, 2, 0).reshape(NCHUNK, 128, BL)))

    in_maps = []
    for c in range(NCORES):
        jg, bg = c % 2, c // 2
        in_maps.append(
            {"wb": wts[jg],
             "xb": np.ascontiguousarray(xcs[bg].transpose(1, 0, 2)),
             "msk": mask}
        )
    return in_maps


def _host_finish(input, results):
    """Gather selected child capsules and unshard over (j, b)."""
    mask = input.sum(axis=2) != 0.0                     # [B, I]
    keyv = np.where(mask, np.arange(I)[None, :], I)
    sidx = np.sort(keyv, axis=1)[:, :NZC]               # [B, NZC]

    ufull = np.empty((B, I, J, N), dtype=np.float32)
    for c in range(NCORES):
        jg, bg = c % 2, c // 2
        uc = np.asarray(results[c]["u"], dtype=np.float32)
        # u columns are chunk-major: [strip, chunk-in-strip, jl, n]
        uc = uc.reshape(16, BL, NCHUNK, JL, N)
        # partition p = 8*il + bl; i = 16*chunk + il
        uc = uc.transpose(1, 2, 0, 3, 4).reshape(BL, I, JL, N)
        ufull[BL * bg: BL * bg + BL, :, JL * jg: JL * jg + JL, :] = uc
    sel = ufull[np.arange(B)[:, None], sidx]            # [B, NZC, J, N]
    return np.ascontiguousarray(sel.transpose(0, 2, 1, 3))  # [B, J, NZC, N]


def run_on_cores(input, W, trace=False, **trace_kwargs):
    from concourse.bass_utils import run_bass_kernel_spmd

    nc = _get_program()
    in_maps = _host_prep(input, W)
    res = run_bass_kernel_spmd(
        nc, in_maps, list(range(NCORES)), trace=trace, **trace_kwargs
    )
    return _host_finish(input, res.results), res


def kernel(input, W):
    out, _ = run_on_cores(input, W)
    return out


if __name__ == "__main__":
    nc = _get_program()
    n_inst = sum(len(b.instructions) for b in nc.m.functions[0].blocks)
    print(f"built OK: {n_inst} instructions")
    from concourse.bass_interp import CoreSim
    import reference as R
    import jax
    with jax.default_device(jax.devices("cpu")[0]):
        inputs = {k: np.asarray(v) for k, v in R.setup_inputs().items()}
    sim = CoreSim(nc)
    sim.assign_tensors(_host_prep(inputs["input"], inputs["W"])[0])
    sim.simulate()
    print(f"sim time: {sim.time} ns")


# revision 55
# speedup vs baseline: 2.8286x; 1.0049x over previous
"""Trainium2 Bass kernel for nn_DenseCapsuleLayer.

Reference computation:
    u_hat[b, j, k, n] = sum_m W[0, j, idx[b,k], n, m] * x[b, idx[b,k], m]
with idx[b, :] the ascending indices of the NZC=1152 non-zero child capsules
of batch b (x is zero elsewhere).

Strategy (8 NeuronCores, 2-way parent-capsule x 4-way batch mesh):
  * Core c owns j in [16*(c%2), 16*(c%2)+16) and b in [8*(c//2), 8*(c//2)+8).
  * Each core computes the DENSE map u_full[b, i, jl, n] for ALL i in bf16
    (x is zero at non-selected i so u_full there is zero and discarded);
    the select/compaction gather over i and the unshard happen on the host.
  * Per 16-wide child-capsule chunk g (i = 16g+il), the PE computes
        out[(il,bl), (jl,n)] = sum_m x[b, 16g+il, m] * W[j, 16g+il, n, m]
    as ONE K=128 bf16 matmul: stationary = [128,128] block-diagonal packing
    of the core's x slice (8 batches), built on device by ONE batched
    broadcast-multiply per strip against a static 0/1 mask (built one strip
    AHEAD so the PE is never gated on the builder); moving = the core's W
    slice pre-transposed to [(il,m), (jl,n)] (256 free columns).
  * Everything over DMA is bf16 (W, x, mask, output) halving HBM bytes;
    PSUM accumulates f32; the PSUM->SBUF copies cast f32->bf16.
  * The CoreSim cost model charges each DMA's transfer to the ISSUING
    engine's queue and queues run concurrently, so the work is spread:
    SP issues most strip in-DMAs (the first three are split across
    SP/ACT/Pool so the pipeline fills fast); the 36 PSUM->SBUF cast
    copies are split DVE/ACT (1/2 per strip — GPSIMD cannot touch PSUM);
    out-DMAs leave per copy-engine region, mostly on the Pool queue.
  * The output staging tiles (o_v/o_a) and the stationaries (bdt) are NOT
    pool-recycled: each strip gets its own SBUF tile, which deletes every
    slot-reuse WAR/WAW hazard (and its dummy-op tax) on the copy engines.
    Only the W stream (3 slots) and PSUM (4 quad slots) recycle.

Toolchain constraints: every lowered instruction accepts ONE sync-wait
command, Tile emits a wait per dependency it cannot prove covered, and its
coverage tracking credits only REAL data dependencies.  Per strip, dummy
ops each carry one wait: sdumv (DVE) reads the fresh W strip so the
stationary-builder needs no DMA wait, and on PE zero-cost bare ldweights
reads absorb the in-DMA (sdum), the stationary RAW (bdtdum), and the
PSUM-slot WAR (gdum reads the o region the bank-freeing copy wrote).
Matmuls may carry a PE self-wait (PSUM slot WAW), copies carry their PE
RAW, out-DMAs carry their copy engine's tick.  SP nops park the
kernel-tail drain's wait list, and a BIR post-pass drops the in-DMA waits
(W-slot WAR) that are provably implied by the kept PE wait.
"""

import numpy as np

B, I, J, M, N = 32, 2304, 32, 8, 16
NZC = I // 2
NCORES = 8
JL = J // 2               # parent capsules per core (16)
JN = JL * N               # 256
BL = B // 4               # batches per core (8)
NCHUNK = I // 16          # 144 chunks of 16 child capsules
NSTRIP = 18
G = NCHUNK // NSTRIP      # 8 chunks per strip
# DVE/ACT copy split alternates (2,6)/(4,4) by strip parity: PSUM tags
# psA0[2]+psA1[4]+psB0[6]+psB1[4] = exactly 8 banks, and the average
# balances DVE (which also builds stationaries) against ACT.
GVP = [4, 2]              # chunks drained by DVE, by strip parity
SJN = G * JN              # 2048 columns per strip in u

V, A, P = "vector", "scalar", "gpsimd"

_CACHE = {}


def _build_program(legalize=True):
    import concourse.bass as bass
    import concourse.mybir as mybir
    import concourse.tile as tile

    f32 = mybir.dt.float32
    bf16 = mybir.dt.bfloat16
    nc = bass.Bass()

    # wb[g, (il,m), 0:256] = W[j, 16g+il, n, m]  (moving operand)
    wb = nc.declare_dram_parameter("wb", [NCHUNK, 128, JN], bf16,
                                   isOutput=False)
    # xb[(il,m), g, bl] = x[bl, 16g+il, m] for the core's 8 batches
    xb = nc.declare_dram_parameter("xb", [128, NCHUNK, BL], bf16,
                                   isOutput=False)
    # msk[(il,m), (il',bl)] = 1.0 iff il == il'
    msk = nc.declare_dram_parameter("msk", [128, 128], bf16, isOutput=False)
    u = nc.declare_dram_parameter("u", [128, NSTRIP * SJN], bf16,
                                  isOutput=True)

    keep_pref = {}   # DMA inst name -> sem prefix of the wait to keep

    with tile.TileContext(nc, pool_alloc_mode="queue") as tc:
        with (
            tc.tile_pool(name="wpool", bufs=1) as wpool,
            tc.tile_pool(name="ppool", bufs=1, space="PSUM") as ppool,
            tc.tile_pool(name="zpool", bufs=1) as zpool,
        ):
            ENG = {V: nc.vector, A: nc.scalar, P: nc.gpsimd, "sync": nc.sync}
            last_on = {}          # engine key -> last op (forced order)
            last_eng = {}         # engine key -> last non-DMA engine op

            def emit(key, op):
                prev = last_on.get(key)
                if prev is not None:
                    tile.add_dep_helper(op.ins, prev.ins, sync=False,
                                        reason="ord")
                last_on[key] = op
                if type(op.ins).__name__ != "InstDMACopy":
                    last_eng[key] = op
                return op

            def ecopy(key, dst, src):
                if key == A:
                    return emit(A, nc.scalar.copy(dst, src))
                return emit(key, ENG[key].tensor_copy(dst, src))

            mask_t = zpool.tile([128, 128], bf16, tag="msk")
            scr_v = zpool.tile([1, 32], bf16, tag="scr0")
            scr_p = zpool.tile([1, 32], bf16, tag="scr1")
            scr_a = zpool.tile([1, 8], bf16, tag="scr2")
            pcol = [0]

            def pcell():
                c = pcol[0]
                pcol[0] += 1
                assert c < 32
                return scr_p[0:1, c: c + 1]
            scol = [0]

            def vcell():
                c = scol[0]
                scol[0] += 1
                assert c < 32
                return scr_v[0:1, c: c + 1]

            x_sb = zpool.tile([128, NCHUNK, BL], bf16, tag="x")
            bdps = []
            # per-strip unpooled tiles (no slot reuse -> no WAR/WAW tax);
            # one o tile per strip: DVE writes cols [0:VJN), ACT the rest,
            # and a single out-DMA moves the whole strip
            o_ts = []
            for s in range(NSTRIP):
                t1 = zpool.tile([128, SJN], bf16, tag=f"o{s}")
                o_ts.append(t1)
            for q in range(NSTRIP // 2):
                t3 = zpool.tile([128, 2 * G, 128], bf16, tag=f"bd{q}")
                bdps.append(t3)

            def bdt_of(s):
                lo = (s % 2) * G
                return bdps[s // 2][:, lo: lo + G, :]

            all_dmas = []

            # --- startup ---------------------------------------------------
            d_msk = emit(P, nc.gpsimd.dma_start(out=mask_t[:, :],
                                                in_=msk[:, :]))
            all_dmas.append(d_msk)
            # x ships once, early, on the Pool queue
            d_x = emit(P, nc.gpsimd.dma_start(out=x_sb[:, :, :],
                                              in_=xb[:, :, :]))
            all_dmas.append(d_x)
            # absorb the mask and x ticks on DVE (the only consumers)
            emit(V, nc.vector.tensor_copy(vcell(), mask_t[0:1, 0:1]))
            emit(V, nc.vector.tensor_copy(vcell(), x_sb[0:1, 0, 0:1]))

            mask4 = mask_t.rearrange("p (s r c) -> p s r c", s=1, r=16)

            w_tiles = []

            def prefetch(s, qk="sync", halves=False):
                w_sb = wpool.tile([128, G, JN], bf16, tag=f"w{s % 6}")
                w_tiles.append(w_sb)
                if halves:
                    h = G // 2
                    for qq, lo in (("sync", 0), (A, h)):
                        d_in = emit(qq, ENG[qq].dma_start(
                            out=w_sb[:, lo: lo + h, :],
                            in_=wb[s * G + lo: s * G + lo + h].rearrange(
                                "g p c -> p g c"),
                        ))
                        keep_pref[d_in.ins.name] = "PE"
                        all_dmas.append(d_in)
                else:
                    d_in = emit(qk, ENG[qk].dma_start(
                        out=w_sb[:, :, :],
                        in_=wb[s * G: (s + 1) * G].rearrange("g p c -> p g c"),
                    ))
                    keep_pref[d_in.ins.name] = "PE"
                    all_dmas.append(d_in)

            def build_bdt(s, n=1):
                """Stationary build for strips s..s+n-1 (one mul)."""
                gg = n * G
                tgt = bdps[s // 2] if n == 2 else bdt_of(s)
                x4 = x_sb[:, s * G: s * G + gg, :].rearrange(
                    "p g (s c) -> p g s c", s=1).broadcast_to([128, gg, 16, BL])
                emit(V, nc.vector.tensor_mul(
                    tgt.rearrange("p g (r c) -> p g r c", r=16),
                    x4,
                    mask4.broadcast_to([128, gg, 16, BL]),
                ))

            # split the first strip's load across two queues so the pipeline
            # fills fast; spread the next two over otherwise-idle queues
            prefetch(0, halves=True)
            prefetch(1, "sync")
            prefetch(2, "sync")
            prefetch(3, "sync")
            # ACT warmup: pays the activation-table load during startup
            # idle, after the startup DMAs ACT issues
            emit(A, nc.scalar.copy(scr_a[0:1, 0:1], x_sb[0:1, 0, 0:1]))
            build_bdt(0, n=2)
            build_bdt(2, n=2)

            for s in range(NSTRIP):
                w_sb = w_tiles[s]
                o_t, bdt = o_ts[s], bdt_of(s)

                # absorb the strip in-DMA tick on PE, then the stationary
                # builder's tick (both zero-cost bare weight loads)
                emit("tensor", nc.tensor.ldweights(w_sb[0:32, 0, 0:1]))
                emit("tensor", nc.tensor.ldweights(bdt[0:32, 0, 0:1]))

                # build upcoming stationaries (a strip-pair per mul) ahead
                # of this strip's DVE copy so the PE is never gated on them
                if s % 2 == 1 and s + 3 < NSTRIP:
                    build_bdt(s + 3, n=2)

                acopies = []
                gv = GVP[s % 2]
                vjn = gv * JN
                for ek, base, gn in ((A, vjn, G - gv), (V, 0, gv)):
                    tagc = "A" if ek == V else "B"
                    ps = ppool.tile([128, gn, JN], f32,
                                    tag=f"ps{tagc}{s % 2}")
                    assert gn == (GVP[s % 2] if ek == V else G - GVP[s % 2])
                    if s >= 2:
                        # gdum: read the o region the bank-freeing copy (same
                        # slot, two strips back) wrote — the cross-engine
                        # wait covers the PSUM-slot WAR
                        emit("tensor", nc.tensor.ldweights(
                            o_ts[s - 2][0:32, base: base + 1]))
                    for h in range(gn):
                        gl = (0 if ek == V else gv) + h
                        emit("tensor", nc.tensor.matmul(
                            ps[:, h, :], bdt[:, gl, :], w_sb[:, gl, 0:JN],
                            start=True, stop=True,
                        ))
                    if ek == V or s < NSTRIP - 1:
                        cp = ecopy(ek, o_t[:, base: base + gn * JN],
                                   ps.rearrange("p a b -> p (a b)"))
                        if ek == A:
                            acopies.append((cp, base, gn * JN))
                    else:
                        # last strip: small 2-chunk ACT copies so the final
                        # copy->out chain is short
                        for j in range(gn // 2):
                            cp = ecopy(A, o_t[:, base + 2 * j * JN:
                                              base + 2 * (j + 1) * JN],
                                       ps[:, 2 * j: 2 * j + 2, :].rearrange(
                                           "p a b -> p (a b)"))
                            acopies.append((cp, base + 2 * j * JN, 2 * JN))

                if s + 4 == NSTRIP - 14 + 4:
                    pass
                if s + 4 < NSTRIP:
                    prefetch(s + 4)

                # pdum: near-free Pool op reading the DVE region — its DVE
                # wait lets the strip out-DMA(s) carry only the ACT wait
                emit(P, nc.gpsimd.tensor_copy(pcell(), o_t[0:1, 0:1]))
                if s in (NSTRIP - 3, NSTRIP - 2):
                    # late strips go out on SP (idle by then) as separate
                    # V/A-region DMAs so each carries one engine wait
                    for base, w, pref in ((0, vjn, "DVE"),
                                          (vjn, SJN - vjn, "Activation")):
                        d_out = emit("sync", nc.sync.dma_start(
                            out=u[:, s * SJN + base: s * SJN + base + w],
                            in_=o_t[:, base: base + w]))
                        keep_pref[d_out.ins.name] = pref
                        all_dmas.append(d_out)
                elif s < NSTRIP - 1:
                    d_out = emit(P, nc.gpsimd.dma_start(
                        out=u[:, s * SJN: (s + 1) * SJN], in_=o_t[:, :]))
                    keep_pref[d_out.ins.name] = "Activation"
                    all_dmas.append(d_out)
                else:
                    # last strip: out-DMA per ACT copy (the first also
                    # carries the DVE region, covered by the pdum); the
                    # later ones issue from ACT itself, which is idle and
                    # needs no cross-engine wait for its own copies
                    for j, (cp, cb, cw) in enumerate(acopies):
                        lo = 0 if j == 0 else cb
                        hi = cb + cw
                        qk = P if j == 0 else A
                        d_out = emit(qk, ENG[qk].dma_start(
                            out=u[:, s * SJN + lo: s * SJN + hi],
                            in_=o_t[:, lo: hi]))
                        keep_pref[d_out.ins.name] = "Activation"
                        all_dmas.append(d_out)

            # tail parking: cover DMAs + engine tails so the kernel-tail
            # drain has at most one wait left.  SWDGE (Pool) DMA sems only
            # have race-free wait values at their FINAL cumulative count, so
            # park just the last Pool DMA per DMASW lane.
            pool_dmas = [d for d in all_dmas
                         if d.ins.engine == ENG[P].engine]
            park_pool = set()
            for lane in range(8):
                lst = pool_dmas[lane::8]
                if lst:
                    park_pool.add(id(lst[-1]))
            parked = [d for d in all_dmas
                      if d.ins.engine != ENG[P].engine
                      or id(d) in park_pool]
            prev = None
            tails = [op for op in (last_eng.get(k) for k in
                                   ("tensor", V, A)) if op is not None]
            for d in parked + tails:
                w = nc.sync.nop(nofuse=True, hint="park")
                tile.add_dep_helper(w.ins, d.ins, sync=True, reason="park")
                if prev is not None:
                    tile.add_dep_helper(w.ins, prev.ins, sync=False,
                                        reason="ord")
                prev = w

    if not legalize:
        return nc

    # Single-wait legalization: for each multi-wait DMA keep the designated
    # wait — the dropped waits are implied by it through the dummy-op
    # ordering chains (the kept tick is only reached after the dropped
    # dependencies completed).
    import concourse.mybir as mybir2

    for blk in nc.m.functions[0].blocks:
        for inst in blk.instructions:
            si = inst.sync_info
            if si is None or not si.on_wait or len(si.on_wait) < 2:
                continue
            if type(inst).__name__ != "InstDMACopy":
                dbg = inst.debug
                raise RuntimeError(
                    f"unexpected multi-wait {inst.name} "
                    f"({type(inst).__name__} engine={inst.engine} "
                    f"line={getattr(dbg, 'lineno', None)}) "
                    f"{[w.ant_name for w in si.on_wait]}"
                )
            pref = keep_pref.get(inst.name)
            if pref is None:
                raise RuntimeError(
                    f"multi-wait DMA {inst.name} with no keep rule: "
                    f"{[w.ant_name for w in si.on_wait]}"
                )
            keep = [w for w in si.on_wait if w.ant_name.startswith(pref)]
            if len(keep) != 1:
                raise RuntimeError(
                    f"{inst.name}: expected one {pref} wait, got "
                    f"{[w.ant_name for w in si.on_wait]}"
                )
            inst.sync_info = mybir2.SyncInfo(
                on_wait=keep, on_update=list(si.on_update or [])
            )
    return nc


def _get_program():
    if "nc" not in _CACHE:
        _CACHE["nc"] = _build_program()
    return _CACHE["nc"]


def _bf16(a):
    import ml_dtypes
    return np.asarray(a, dtype=np.float32).astype(ml_dtypes.bfloat16)


def _host_prep(input, W):
    """Build per-core in_maps. input: [B, I, M]; W: [1, J, I, N, M]."""
    x = np.ascontiguousarray(input, dtype=np.float32)
    W0 = np.ascontiguousarray(W[0], dtype=np.float32)  # [J, I, N, M]

    # mask[(il, m), (il', bl)] = 1 iff il == il'
    il_row = (np.arange(128) // M)[:, None]
    il_col = (np.arange(128) // BL)[None, :]
    mask = _bf16((il_row == il_col).astype(np.float32))

    wts = []
    for jg in range(2):
        ws = W0[JL * jg: JL * jg + JL]                  # [JL, I, N, M]
        wts.append(_bf16(ws.transpose(1, 3, 0, 2).reshape(NCHUNK, 128, JN)))
    xcs = []
    for bg in range(4):
        xs = x[BL * bg: BL * bg + BL]                   # [BL, I, M]
        # xc[g, (il, m), bl] = x[bl, 16g+il, m]
        xcs.append(_bf16(xs.transpose(1, 2, 0).reshape(NCHUNK, 128, BL)))

    in_maps = []
    for c in range(NCORES):
        jg, bg = c % 2, c // 2
        in_maps.append(
            {"wb": wts[jg],
             "xb": np.ascontiguousarray(xcs[bg].transpose(1, 0, 2)),
             "msk": mask}
        )
    return in_maps


def _host_finish(input, results):
    """Gather selected child capsules and unshard over (j, b)."""
    mask = input.sum(axis=2) != 0.0                     # [B, I]
    keyv = np.where(mask, np.arange(I)[None, :], I)
    sidx = np.sort(keyv, axis=1)[:, :NZC]               # [B, NZC]

    ufull = np.empty((B, I, J, N), dtype=np.float32)
    for c in range(NCORES):
        jg, bg = c % 2, c // 2
        uc = np.asarray(results[c]["u"], dtype=np.float32)
        # u columns are chunk-major: [strip, chunk-in-strip, jl, n]
        uc = uc.reshape(16, BL, NCHUNK, JL, N)
        # partition p = 8*il + bl; i = 16*chunk + il
        uc = uc.transpose(1, 2, 0, 3, 4).reshape(BL, I, JL, N)
        ufull[BL * bg: BL * bg + BL, :, JL * jg: JL * jg + JL, :] = uc
    sel = ufull[np.arange(B)[:, None], sidx]            # [B, NZC, J, N]
    return np.ascontiguousarray(sel.transpose(0, 2, 1, 3))  # [B, J, NZC, N]


def run_on_cores(input, W, trace=False, **trace_kwargs):
    from concourse.bass_utils import run_bass_kernel_spmd

    nc = _get_program()
    in_maps = _host_prep(input, W)
    res = run_bass_kernel_spmd(
        nc, in_maps, list(range(NCORES)), trace=trace, **trace_kwargs
    )
    return _host_finish(input, res.results), res


def kernel(input, W):
    out, _ = run_on_cores(input, W)
    return out


if __name__ == "__main__":
    nc = _get_program()
    n_inst = sum(len(b.instructions) for b in nc.m.functions[0].blocks)
    print(f"built OK: {n_inst} instructions")
    from concourse.bass_interp import CoreSim
    import reference as R
    import jax
    with jax.default_device(jax.devices("cpu")[0]):
        inputs = {k: np.asarray(v) for k, v in R.setup_inputs().items()}
    sim = CoreSim(nc)
    sim.assign_tensors(_host_prep(inputs["input"], inputs["W"])[0])
    sim.simulate()
    print(f"sim time: {sim.time} ns")
